# revision 1
# baseline (speedup 1.0000x reference)
"""Trainium2 Bass kernel for DistributedAFNO2D.

Problem: x(2,768,256,256) f32; per-block (8 blocks of 96 ch) spectral MLP:
  out = irfft2( softshrink( W2*relu(W1*rfft2(x) + b1) + b2 ) ) + x
Block-diagonal channel mixing with shared-per-(u,v) complex 96x96 weights.

Sharding: block k -> core k (8 cores). No collectives. Each core handles
(2, 96, 256, 256) with its own block weights.

All FFTs are dense matmuls with probed DFT matrices (bf16 inputs, fp32 PSUM).
Dataflow per core, per batch b:
  Phase A (per channel c):
    S1 contract h: psY[w_chunk, (Yr-u256 | Yi-u256)] = x[h,w].T-matmul CHpack
    S2 contract w: psZ[u_chunk, (Zr-v129 | Zi-v129)] via R1/R2 consts
    -> Zbuf[b, u, c, 258] bf16 in DRAM
  Phase B (per pair of u):
    Z1 tile [98, 2, 258] (rows 96/97 = bias ones-pattern)
    mix1 (3 matmuls: W1R_aug*Z1 + split-sign W1I on swapped halves) + b1 (K-aug)
    relu (ACT) -> o1P [98, 2, 258] (rows 96/97 ones-pattern)
    mix2 (3 matmuls) + b2 -> psum t
    softshrink: c=clamp(t,+-lam); s=t-c (DVE) -> Sbuf[b, c, u, 258] bf16
  Phase C (per channel c):
    Q^T (v 1..128 on partitions) = s-slices x CHIpack matmuls; combines (DVE)
    DC term q0 = (CHIr.sr0 - CHIi.si0)/16 (N=1 matmuls)
    out[h,w] = QrT.Gc + QiT.Gs (matmuls) + x + q0  (DVE stt)
"""
import os
import sys
import numpy as np

sys.path.insert(0, "/opt/trn_rl_repo")

import ml_dtypes

BF16 = ml_dtypes.bfloat16

H = 256
W = 256
NV = W // 2 + 1  # 129
BLK = 96
NCORES = 8
B = 2
LAM = 0.01


def make_host_consts():
    """All packed constant matrices (numpy bf16) via probing np.fft."""
    I = np.eye(H, dtype=np.float64)
    F = np.fft.fft(I, axis=0, norm='ortho')       # F[u,h]; F@x = fft(x)
    Fi = np.fft.ifft(I, axis=0, norm='ortho')     # Fi[h,u]
    CHr = F.real.T.copy()                          # [h,u]
    CHi = F.imag.T.copy()
    EWr = F.real.T[:, :NV].copy()                  # [w,v]
    EWi = F.imag.T[:, :NV].copy()
    CHIr = Fi.real.T.copy()                        # [u,h]
    CHIi = Fi.imag.T.copy()
    Ir = np.eye(NV)
    Gc = np.fft.irfft(Ir, n=W, axis=-1, norm='ortho')        # [v,w]
    Gs = np.fft.irfft(1j * Ir, n=W, axis=-1, norm='ortho')   # [v,w]

    c = {}
    # CHpack [2][128, 512]: rows h (chunk), cols [CHr-u | CHi-u]
    c['chpack'] = np.stack([
        np.concatenate([CHr[j * 128:(j + 1) * 128, :], CHi[j * 128:(j + 1) * 128, :]], axis=1)
        for j in range(2)])
    # R1 [2][128, 258] = [EWr | EWi]; R2 = [-EWi | EWr] rows w chunk
    c['r1'] = np.stack([
        np.concatenate([EWr[j * 128:(j + 1) * 128], EWi[j * 128:(j + 1) * 128]], axis=1)
        for j in range(2)])
    c['r2'] = np.stack([
        np.concatenate([-EWi[j * 128:(j + 1) * 128], EWr[j * 128:(j + 1) * 128]], axis=1)
        for j in range(2)])
    # CHIpack [2][128, 512]: rows u chunk, cols [CHIr-h | CHIi-h]
    c['chipack'] = np.stack([
        np.concatenate([CHIr[j * 128:(j + 1) * 128], CHIi[j * 128:(j + 1) * 128]], axis=1)
        for j in range(2)])
    # NCHI [2][128, 256] = -CHIi rows u chunk
    c['nchi'] = np.stack([-CHIi[j * 128:(j + 1) * 128] for j in range(2)])
    # G tiles rows v=1..128
    c['gc'] = Gc[1:129]
    c['gs'] = Gs[1:129]
    return {k: v.astype(BF16) for k, v in c.items()}


def make_weight_consts(w1k, b1k, w2k, b2k):
    """Augmented weight matrices for one block.
    w1k/w2k: (96, 96, 2) [i, o, ri]; b1k/b2k: (96, 2) [o, ri]."""
    return {
        'w1r': w1k[..., 0].astype(BF16),
        'w1i': w1k[..., 1].astype(BF16),
        'w1in': (-w1k[..., 1]).astype(BF16),
        'w2r': w2k[..., 0].astype(BF16),
        'w2i': w2k[..., 1].astype(BF16),
        'w2in': (-w2k[..., 1]).astype(BF16),
    }


def build_nc():
    import concourse.bass as bass
    import concourse.tile as tile
    from concourse import bacc, mybir

    dt = mybir.dt
    nc = bacc.Bacc("TRN2", target_bir_lowering=False, debug=False)

    # I/O
    x32 = nc.dram_tensor("x32", [B, BLK, H, W], dt.float32, kind="ExternalInput").ap()
    xbf = nc.dram_tensor("xbf", [B, BLK, H, W], dt.bfloat16, kind="ExternalInput").ap()
    chpack = nc.dram_tensor("chpack", [2, 128, 512], dt.bfloat16, kind="ExternalInput").ap()
    r1 = nc.dram_tensor("r1", [2, 128, 258], dt.bfloat16, kind="ExternalInput").ap()
    r2 = nc.dram_tensor("r2", [2, 128, 258], dt.bfloat16, kind="ExternalInput").ap()
    chipack = nc.dram_tensor("chipack", [2, 128, 512], dt.bfloat16, kind="ExternalInput").ap()
    nchi = nc.dram_tensor("nchi", [2, 128, 256], dt.bfloat16, kind="ExternalInput").ap()
    gc = nc.dram_tensor("gc", [128, 256], dt.bfloat16, kind="ExternalInput").ap()
    gs = nc.dram_tensor("gs", [128, 256], dt.bfloat16, kind="ExternalInput").ap()
    wts = {n: nc.dram_tensor(n, [96, 96], dt.bfloat16, kind="ExternalInput").ap()
           for n in ['w1r', 'w1i', 'w1in', 'w2r', 'w2i', 'w2in']}
    b1cols = nc.dram_tensor("b1cols", [96, 2], dt.float32, kind="ExternalInput").ap()
    b2cols = nc.dram_tensor("b2cols", [96, 4], dt.float32, kind="ExternalInput").ap()
    out = nc.dram_tensor("out", [B, BLK, H, W], dt.float32, kind="ExternalOutput").ap()

    # DRAM scratch
    zbuf = nc.dram_tensor("zbuf", [B, H, BLK, 258], dt.bfloat16).ap()
    sbuf_d = nc.dram_tensor("sbufd", [B, BLK, H, 258], dt.bfloat16).ap()


    with tile.TileContext(nc) as tc:
        from contextlib import ExitStack
        with ExitStack() as ctx:
            consts = ctx.enter_context(tc.tile_pool(name="consts", bufs=1))
            pa_x = ctx.enter_context(tc.tile_pool(name="pa_x", bufs=4))
            pa_y = ctx.enter_context(tc.tile_pool(name="pa_y", bufs=4))
            pa_z = ctx.enter_context(tc.tile_pool(name="pa_z", bufs=4))
            pb_s = ctx.enter_context(tc.tile_pool(name="pb_s", bufs=4))
            pc_in = ctx.enter_context(tc.tile_pool(name="pc_in", bufs=4))
            pc_q = ctx.enter_context(tc.tile_pool(name="pc_q", bufs=4))
            pc_o = ctx.enter_context(tc.tile_pool(name="pc_o", bufs=4))
            # Single PSUM pool: 3 shared tags x (3+3+2) bufs x 1 bank = 8 banks
            psum = ctx.enter_context(tc.tile_pool(name="psum", bufs=1, space="PSUM"))

            # ---- Load constants (one [128, X] tile per chunk) ----
            def chunked_const(name, ap_, ncols):
                ts = []
                for j in range(2):
                    t = consts.tile([128, ncols], dt.bfloat16, tag=f"{name}{j}", name=f"{name}{j}")
                    nc.sync.dma_start(out=t, in_=ap_[j])
                    ts.append(t)
                return ts

            t_ch = chunked_const("t_ch", chpack, 512)
            t_r1 = chunked_const("t_r1", r1, 258)
            t_r2 = chunked_const("t_r2", r2, 258)
            t_chi = chunked_const("t_chi", chipack, 512)
            t_nchi = chunked_const("t_nchi", nchi, 256)
            t_gc = consts.tile([128, 256], dt.bfloat16, tag="t_gc", name="t_gc")
            nc.sync.dma_start(out=t_gc, in_=gc)
            t_gs = consts.tile([128, 256], dt.bfloat16, tag="t_gs", name="t_gs")
            nc.sync.dma_start(out=t_gs, in_=gs)
            t_w = {}
            for n, ap_ in wts.items():
                t_w[n] = consts.tile([96, 96], dt.bfloat16, tag=f"t_{n}", name=f"t_{n}")
                nc.sync.dma_start(out=t_w[n], in_=ap_)

            t_b2 = consts.tile([96, 4], dt.float32, tag="t_b2", name="t_b2")
            nc.sync.dma_start(out=t_b2, in_=b2cols)
            t_b1 = consts.tile([96, 2], dt.float32, tag="t_b1", name="t_b1")
            nc.sync.dma_start(out=t_b1, in_=b1cols)

            for b in range(B):
                # ================= Phase A =================
                for c in range(BLK):
                    xt0 = pa_x.tile([128, 256], dt.bfloat16, tag="xt0", name="xt0")
                    nc.sync.dma_start(out=xt0, in_=xbf[b, c, 0:128, :])
                    xt1 = pa_x.tile([128, 256], dt.bfloat16, tag="xt1", name="xt1")
                    nc.sync.dma_start(out=xt1, in_=xbf[b, c, 128:256, :])

                    ys = []
                    for wc in range(2):
                        psy = psum.tile([128, 512], dt.float32, tag="psA", name="psy", bufs=3)
                        nc.tensor.matmul(psy, lhsT=xt0[:, wc * 128:(wc + 1) * 128],
                                         rhs=t_ch[0], start=True, stop=False)
                        nc.tensor.matmul(psy, lhsT=xt1[:, wc * 128:(wc + 1) * 128],
                                         rhs=t_ch[1], start=False, stop=True)
                        y = pa_y.tile([128, 512], dt.bfloat16, tag=f"y{wc}", name=f"y{wc}")
                        nc.scalar.copy(y, psy)
                        ys.append(y)

                    for uc in range(2):
                        psz = psum.tile([128, 512], dt.float32, tag="psB", name="psz", bufs=3)
                        us = slice(uc * 128, (uc + 1) * 128)
                        us2 = slice(256 + uc * 128, 256 + (uc + 1) * 128)
                        nc.tensor.matmul(psz[:, 0:258], lhsT=ys[0][:, us], rhs=t_r1[0], start=True, stop=False)
                        nc.tensor.matmul(psz[:, 0:258], lhsT=ys[0][:, us2], rhs=t_r2[0], start=False, stop=False)
                        nc.tensor.matmul(psz[:, 0:258], lhsT=ys[1][:, us], rhs=t_r1[1], start=False, stop=False)
                        nc.tensor.matmul(psz[:, 0:258], lhsT=ys[1][:, us2], rhs=t_r2[1], start=False, stop=True)
                        zt = pa_z.tile([128, 258], dt.bfloat16, tag="zt", name="zt")
                        nc.scalar.copy(zt, psz[:, 0:258])
                        nc.sync.dma_start(out=zbuf[b, uc * 128:(uc + 1) * 128, c, :], in_=zt)

                # ================= Phase B =================
                for u in range(H):
                    z1 = pb_s.tile([96, 258], dt.bfloat16, tag="z1", name="z1")
                    nc.gpsimd.dma_start(out=z1, in_=zbuf[b, u, :, :])

                    ps1 = psum.tile([96, 512], dt.float32, tag="psA", name="ps1", bufs=3)
                    nc.tensor.matmul(ps1[:, 0:258], lhsT=t_w['w1r'], rhs=z1, start=True, stop=False,
                                     skip_group_check=True)
                    nc.tensor.matmul(ps1[:, 0:129], lhsT=t_w['w1in'], rhs=z1[:, 129:258],
                                     start=False, stop=True, skip_group_check=True)
                    nc.tensor.matmul(ps1[:, 129:258], lhsT=t_w['w1i'], rhs=z1[:, 0:129],
                                     start=False, stop=True, skip_group_check=True)

                    o1 = pb_s.tile([96, 258], dt.bfloat16, tag="o1", name="o1")
                    nc.scalar.activation(o1[:, 0:129], ps1[:, 0:129],
                                         mybir.ActivationFunctionType.Relu, bias=t_b1[:, 0:1])
                    nc.scalar.activation(o1[:, 129:258], ps1[:, 129:258],
                                         mybir.ActivationFunctionType.Relu, bias=t_b1[:, 1:2])

                    ps2 = psum.tile([96, 512], dt.float32, tag="psB", name="ps2", bufs=3)
                    nc.tensor.matmul(ps2[:, 0:258], lhsT=t_w['w2r'], rhs=o1, start=True, stop=False,
                                     skip_group_check=True)
                    nc.tensor.matmul(ps2[:, 0:129], lhsT=t_w['w2in'], rhs=o1[:, 129:258],
                                     start=False, stop=True, skip_group_check=True)
                    nc.tensor.matmul(ps2[:, 129:258], lhsT=t_w['w2i'], rhs=o1[:, 0:129],
                                     start=False, stop=True, skip_group_check=True)

                    # softshrink with b2 folded into clamp bounds:
                    # s = o2 - clamp(o2, -lam-b2, lam-b2)
                    cl = pb_s.tile([96, 258], dt.float32, tag="cl", name="cl")
                    nc.vector.tensor_scalar(cl[:, 0:129], ps2[:, 0:129], t_b2[:, 0:1], t_b2[:, 1:2],
                                            mybir.AluOpType.min, mybir.AluOpType.max)
                    nc.vector.tensor_scalar(cl[:, 129:258], ps2[:, 129:258], t_b2[:, 2:3], t_b2[:, 3:4],
                                            mybir.AluOpType.min, mybir.AluOpType.max)
                    st = pb_s.tile([96, 258], dt.bfloat16, tag="st", name="st")
                    nc.vector.tensor_tensor(st, ps2[:, 0:258], cl, mybir.AluOpType.subtract)
                    nc.sync.dma_start(out=sbuf_d[b, :, u, :], in_=st)

                # ================= Phase C =================
                for c in range(BLK):
                    st0 = pc_in.tile([128, 258], dt.bfloat16, tag="st0", name="st0")
                    nc.gpsimd.dma_start(out=st0, in_=sbuf_d[b, c, 0:128, :])
                    st1 = pc_in.tile([128, 258], dt.bfloat16, tag="st1", name="st1")
                    nc.gpsimd.dma_start(out=st1, in_=sbuf_d[b, c, 128:256, :])

                    # QrT = sr.CHIr - si.CHIi ; QiT = sr.CHIi + si.CHIr  (psum accum)
                    psa = psum.tile([128, 256], dt.float32, tag="psA", name="psa", bufs=3)
                    nc.tensor.matmul(psa, lhsT=st0[:, 1:129], rhs=t_chi[0][:, 0:256], start=True, stop=False)
                    nc.tensor.matmul(psa, lhsT=st1[:, 1:129], rhs=t_chi[1][:, 0:256], start=False, stop=False)
                    nc.tensor.matmul(psa, lhsT=st0[:, 130:258], rhs=t_nchi[0], start=False, stop=False)
                    nc.tensor.matmul(psa, lhsT=st1[:, 130:258], rhs=t_nchi[1], start=False, stop=True)
                    psb = psum.tile([128, 256], dt.float32, tag="psB", name="psb", bufs=3)
                    nc.tensor.matmul(psb, lhsT=st0[:, 1:129], rhs=t_chi[0][:, 256:512], start=True, stop=False)
                    nc.tensor.matmul(psb, lhsT=st1[:, 1:129], rhs=t_chi[1][:, 256:512], start=False, stop=False)
                    nc.tensor.matmul(psb, lhsT=st0[:, 130:258], rhs=t_chi[0][:, 0:256], start=False, stop=False)
                    nc.tensor.matmul(psb, lhsT=st1[:, 130:258], rhs=t_chi[1][:, 0:256], start=False, stop=True)

                    qr = pc_q.tile([128, 256], dt.bfloat16, tag="qr", name="qr")
                    nc.scalar.copy(qr, psa)
                    qi = pc_q.tile([128, 256], dt.bfloat16, tag="qi", name="qi")
                    nc.scalar.copy(qi, psb)

                    # DC (v=0) term -> q0 per h-chunk
                    psq = psum.tile([128, 2], dt.float32, tag="psC", name="psq", bufs=2)
                    for hc in range(2):
                        hs = slice(hc * 128, (hc + 1) * 128)
                        nc.tensor.matmul(psq[:, hc:hc + 1], lhsT=t_chi[0][:, hs], rhs=st0[:, 0:1],
                                         start=(hc == 0), stop=False, skip_group_check=True)
                        nc.tensor.matmul(psq[:, hc:hc + 1], lhsT=t_nchi[0][:, hs], rhs=st0[:, 129:130],
                                         start=False, stop=False, skip_group_check=True)
                        nc.tensor.matmul(psq[:, hc:hc + 1], lhsT=t_chi[1][:, hs], rhs=st1[:, 0:1],
                                         start=False, stop=False, skip_group_check=True)
                        nc.tensor.matmul(psq[:, hc:hc + 1], lhsT=t_nchi[1][:, hs], rhs=st1[:, 129:130],
                                         start=False, stop=True, skip_group_check=True)
                    q0 = pc_q.tile([128, 2], dt.float32, tag="q0", name="q0")
                    nc.vector.tensor_scalar_mul(q0, psq, 1.0 / 16.0)

                    for hc in range(2):
                        hs = slice(hc * 128, (hc + 1) * 128)
                        pso = psum.tile([128, 512], dt.float32, tag="psC", name="pso", bufs=2)[:, 0:256]
                        nc.tensor.matmul(pso, lhsT=qr[:, hs], rhs=t_gc, start=True, stop=False)
                        nc.tensor.matmul(pso, lhsT=qi[:, hs], rhs=t_gs, start=False, stop=True)
                        xt = pc_o.tile([128, 256], dt.float32, tag="xt", name="xt")
                        nc.sync.dma_start(out=xt, in_=x32[b, c, hs, :])
                        ot = pc_o.tile([128, 256], dt.float32, tag="ot", name="ot")
                        nc.vector.scalar_tensor_tensor(
                            ot, xt, q0[:, hc:hc + 1], pso,
                            mybir.AluOpType.add, mybir.AluOpType.add)
                        nc.sync.dma_start(out=out[b, c, hs, :], in_=ot)
    nc.compile()
    return nc


_NC_CACHE = {}


def _get_nc():
    if 'nc' not in _NC_CACHE:
        _NC_CACHE['nc'] = build_nc()
    return _NC_CACHE['nc']


def make_in_maps(x, w1, b1, w2, b2):
    hc = make_host_consts()
    x = np.ascontiguousarray(x, dtype=np.float32)
    in_maps = []
    for k in range(NCORES):
        xk = np.ascontiguousarray(x[:, BLK * k:BLK * (k + 1)])
        wk = make_weight_consts(w1[k], b1[k, :, 0, 0, :], w2[k], b2[k, :, 0, 0, :])
        b2k = b2[k, :, 0, 0, :]
        b2cols = np.stack([LAM - b2k[:, 0], -LAM - b2k[:, 0],
                           LAM - b2k[:, 1], -LAM - b2k[:, 1]], axis=1).astype(np.float32)
        b1cols = np.ascontiguousarray(b1[k, :, 0, 0, :], dtype=np.float32)
        m = dict(
            b1cols=b1cols,
            b2cols=b2cols,
            x32=xk,
            xbf=xk.astype(BF16),
            chpack=hc['chpack'], r1=hc['r1'], r2=hc['r2'],
            chipack=hc['chipack'], nchi=hc['nchi'], gc=hc['gc'], gs=hc['gs'],
            **wk,
        )
        in_maps.append(m)
    return in_maps


def kernel(x, w1, b1, w2, b2):
    from concourse.bass_utils import run_bass_kernel_spmd
    nc = _get_nc()
    in_maps = make_in_maps(np.asarray(x), np.asarray(w1), np.asarray(b1),
                           np.asarray(w2), np.asarray(b2))
    res = run_bass_kernel_spmd(nc, in_maps, core_ids=list(range(NCORES)))
    outs = [res.results[k]['out'] for k in range(NCORES)]
    return np.concatenate(outs, axis=1)



# revision 4
# speedup vs baseline: 1.1830x; 1.1830x over previous
"""Trainium2 Bass kernel for DistributedAFNO2D (v2).

Problem: x(2,768,256,256) f32; per-block (8 blocks of 96 ch) spectral MLP:
  out = irfft2( softshrink( W2*relu(W1*rfft2(x) + b1) + b2 ) ) + x
Sharding: block k -> core k (8 cores). No collectives.

v2 design vs v1 (1.88ms):
 - Phase A (fwd FFT): S1 computes only u=0..128 (rfft symmetry), S2 derives
   the mirror half u=255..128 from conj(Y) with sign-flipped EW consts.
   Mirror rows live in their own DRAM buffer (zbufM, rows j <-> u=255-j);
   all reversals are absorbed into host-built constant row orders.
 - Phase B (spectral MLP): weight-stationary matmuls (96-col LDW hides under
   N=387 moving), 3 u-rows per group, re/im in separate PSUM tiles.
   softshrink = relu(t+b2-lam) + min(t+b2+lam, 0) via ACT+DVE.
 - Phase C (inv FFT): DC(v=0) folded into the main matmuls (P1 rows=Qr[0..127]
   with Gc[0]=1/16 row); Nyquist v=128 folded via P2 row127=-Qr[128] paired
   with -Gc[128] in GSX. No tiny DC matmuls, no scalar_tensor_tensor.
 - Per-batch DRAM tensors + zipped emission (A0 | B0+A1 | C0+B1 | C1) keep
   the PE warm; x stays resident in SBUF for the residual add.
 - Output bf16, upcast on host.

sd column layout (C1 lhsT windows, per u-row):
  [ s_re v0..127 | s_im v128 | s_im v0..127 | -s_re v128 ]
   P1-A=0:128 (x)CHIr ; P1-B=129:257 (x)-CHIi -> rows Qr v0..127
   P2-A=1:129 (x)CHIi ; P2-B=130:258 (x)CHIr  -> rows Qi v1..127, row127=-Qr[128]
"""
import sys
import numpy as np

sys.path.insert(0, "/opt/trn_rl_repo")

import ml_dtypes

BF16 = ml_dtypes.bfloat16

H = 256
W = 256
NV = 129
BLK = 96
NCORES = 8
B = 2
LAM = 0.01
U = 3  # u-rows per phase-B group
CG = 4  # channels per DMA batch


def b_groups():
    """(u0, cnt) groups covering 128 rows."""
    out = []
    u0 = 0
    while u0 < 128:
        cnt = min(U, 128 - u0)
        out.append((u0, cnt))
        u0 += cnt
    return out


def make_host_consts():
    I = np.eye(H, dtype=np.float64)
    F = np.fft.fft(I, axis=0, norm='ortho')       # F[u,h]
    Fi = np.fft.ifft(I, axis=0, norm='ortho')     # Fi[h,u]
    CHr = F.real.T.copy()                          # [h,u]
    CHi = F.imag.T.copy()
    EWr = F.real.T[:, :NV].copy()                  # [w,v]
    EWi = F.imag.T[:, :NV].copy()
    CHIr = Fi.real.T.copy()                        # [u,h]
    CHIi = Fi.imag.T.copy()
    Ir = np.eye(NV)
    Gc = np.fft.irfft(Ir, n=W, axis=-1, norm='ortho')        # [v,w]
    Gs = np.fft.irfft(1j * Ir, n=W, axis=-1, norm='ortho')   # [v,w]

    c = {}
    # A1: [h-chunk, (CHr u0..128 | CHi u0..128)]
    c['chh'] = np.stack([
        np.concatenate([CHr[j * 128:(j + 1) * 128, 0:NV],
                        CHi[j * 128:(j + 1) * 128, 0:NV]], axis=1)
        for j in range(2)])
    # A2 direct: R1 = [EWr|EWi], R2 = [-EWi|EWr]; mirror: R2m = [EWi|-EWr]
    c['r1'] = np.stack([
        np.concatenate([EWr[j * 128:(j + 1) * 128], EWi[j * 128:(j + 1) * 128]], axis=1)
        for j in range(2)])
    c['r2'] = np.stack([
        np.concatenate([-EWi[j * 128:(j + 1) * 128], EWr[j * 128:(j + 1) * 128]], axis=1)
        for j in range(2)])
    c['r2m'] = (-c['r2']).copy()
    # C1 rhs consts: direct rows u=0..127; mirror rows j <-> u=255-j
    perm = np.array([255 - j for j in range(128)])
    c['cr0'] = CHIr[0:128]
    c['ci0'] = CHIi[0:128]
    c['ni0'] = -CHIi[0:128]
    c['crM'] = CHIr[perm]
    c['ciM'] = CHIi[perm]
    c['niM'] = -CHIi[perm]
    # C2: GCX rows v=0..127 (incl DC); GSX rows = [Gs v1..127 ; -Gc v128]
    c['gcx'] = Gc[0:128]
    c['gsx'] = np.concatenate([Gs[1:128], -Gc[128:129]], axis=0)
    return {k: v.astype(BF16) for k, v in c.items()}


def build_nc():
    import concourse.bass as bass
    import concourse.tile as tile
    from concourse import bacc, mybir

    dt = mybir.dt
    Alu = mybir.AluOpType
    Act = mybir.ActivationFunctionType
    nc = bacc.Bacc("TRN2", target_bir_lowering=False, debug=False)

    # ---- I/O ----
    xbf = nc.dram_tensor("xbf", [B, BLK, H, W], dt.bfloat16, kind="ExternalInput").ap()
    chh = nc.dram_tensor("chh", [2, 128, 258], dt.bfloat16, kind="ExternalInput").ap()
    r1 = nc.dram_tensor("r1", [2, 128, 258], dt.bfloat16, kind="ExternalInput").ap()
    r2 = nc.dram_tensor("r2", [2, 128, 258], dt.bfloat16, kind="ExternalInput").ap()
    r2m = nc.dram_tensor("r2m", [2, 128, 258], dt.bfloat16, kind="ExternalInput").ap()
    cr0 = nc.dram_tensor("cr0", [128, 256], dt.bfloat16, kind="ExternalInput").ap()
    ci0 = nc.dram_tensor("ci0", [128, 256], dt.bfloat16, kind="ExternalInput").ap()
    ni0 = nc.dram_tensor("ni0", [128, 256], dt.bfloat16, kind="ExternalInput").ap()
    crM = nc.dram_tensor("crM", [128, 256], dt.bfloat16, kind="ExternalInput").ap()
    ciM = nc.dram_tensor("ciM", [128, 256], dt.bfloat16, kind="ExternalInput").ap()
    niM = nc.dram_tensor("niM", [128, 256], dt.bfloat16, kind="ExternalInput").ap()
    gcx = nc.dram_tensor("gcx", [128, 256], dt.bfloat16, kind="ExternalInput").ap()
    gsx = nc.dram_tensor("gsx", [128, 256], dt.bfloat16, kind="ExternalInput").ap()
    wts = {n: nc.dram_tensor(n, [96, 96], dt.bfloat16, kind="ExternalInput").ap()
           for n in ['w1r', 'w1i', 'w1in', 'w2r', 'w2i', 'w2in']}
    b1c = nc.dram_tensor("b1c", [96, 2], dt.float32, kind="ExternalInput").ap()
    bAc = nc.dram_tensor("bAc", [96, 2], dt.float32, kind="ExternalInput").ap()  # b2-lam
    bMc = nc.dram_tensor("bMc", [96, 2], dt.float32, kind="ExternalInput").ap()  # b2+lam
    outd = nc.dram_tensor("out", [B, BLK, H, W], dt.bfloat16, kind="ExternalOutput").ap()

    # DRAM scratch, separate tensors per batch to avoid cross-batch false deps
    zD = [nc.dram_tensor(f"zD{b}", [BLK, 128, 258], dt.bfloat16).ap() for b in range(B)]
    zM = [nc.dram_tensor(f"zM{b}", [BLK, 128, 258], dt.bfloat16).ap() for b in range(B)]
    sD = [nc.dram_tensor(f"sD{b}", [BLK, 128, 258], dt.bfloat16).ap() for b in range(B)]
    sM = [nc.dram_tensor(f"sM{b}", [BLK, 128, 258], dt.bfloat16).ap() for b in range(B)]

    NG = BLK // CG  # 24 channel groups

    with tile.TileContext(nc) as tc:
        from contextlib import ExitStack
        with ExitStack() as ctx:
            consts = ctx.enter_context(tc.tile_pool(name="consts", bufs=1))
            xres = ctx.enter_context(tc.tile_pool(name="xres", bufs=1))
            pa = ctx.enter_context(tc.tile_pool(name="pa", bufs=3))
            pb = ctx.enter_context(tc.tile_pool(name="pb", bufs=3))
            pc = ctx.enter_context(tc.tile_pool(name="pc", bufs=3))
            psum = ctx.enter_context(tc.tile_pool(name="psum", bufs=2, space="PSUM"))

            # ---- constants ----
            def ld(name, ap_, shape):
                t = consts.tile(shape, dt.bfloat16, tag=name, name=name)
                nc.sync.dma_start(out=t, in_=ap_)
                return t

            t_chh = [ld(f"chh{j}", chh[j], [128, 258]) for j in range(2)]
            t_r1 = [ld(f"r1{j}", r1[j], [128, 258]) for j in range(2)]
            t_r2 = [ld(f"r2{j}", r2[j], [128, 258]) for j in range(2)]
            t_r2m = [ld(f"r2m{j}", r2m[j], [128, 258]) for j in range(2)]
            t_cr0 = ld("cr0", cr0, [128, 256])
            t_ci0 = ld("ci0", ci0, [128, 256])
            t_ni0 = ld("ni0", ni0, [128, 256])
            t_crM = ld("crM", crM, [128, 256])
            t_ciM = ld("ciM", ciM, [128, 256])
            t_niM = ld("niM", niM, [128, 256])
            t_gcx = ld("gcx", gcx, [128, 256])
            t_gsx = ld("gsx", gsx, [128, 256])
            t_w = {n: ld(n, ap_, [96, 96]) for n, ap_ in wts.items()}
            t_b1 = consts.tile([96, 2], dt.float32, tag="b1", name="t_b1")
            nc.sync.dma_start(out=t_b1, in_=b1c)
            t_bA = consts.tile([96, 2], dt.float32, tag="bA", name="t_bA")
            nc.sync.dma_start(out=t_bA, in_=bAc)
            t_bM = consts.tile([96, 2], dt.float32, tag="bM", name="t_bM")
            nc.sync.dma_start(out=t_bM, in_=bMc)

            # x resident tiles, one per (channel-group, h-chunk), reused across batches
            xr = [[xres.tile([128, CG, 256], dt.bfloat16, tag=f"xr{g}_{hc}",
                             name=f"xr{g}_{hc}") for hc in range(2)]
                  for g in range(NG)]

            # =================== Phase A ===================
            def emit_A(b):
                for g in range(NG):
                    c4 = g * CG
                    for hc in range(2):
                        nc.sync.dma_start(
                            out=xr[g][hc],
                            in_=xbf[b, c4:c4 + CG, hc * 128:(hc + 1) * 128, :]
                            .transpose([1, 0, 2]))
                    ztD = pa.tile([128, CG, 258], dt.bfloat16, tag="ztD", name="ztD", bufs=2)
                    ztM = pa.tile([128, CG, 258], dt.bfloat16, tag="ztM", name="ztM", bufs=2)
                    for cl in range(CG):
                        ys = []
                        for wc in range(2):
                            psY = psum.tile([128, 258], dt.float32, tag="pX1",
                                            name="psY", bufs=2)
                            for hc in range(2):
                                nc.tensor.matmul(
                                    psY, lhsT=xr[g][hc][:, cl, wc * 128:(wc + 1) * 128],
                                    rhs=t_chh[hc], start=(hc == 0), stop=(hc == 1))
                            y = pa.tile([128, 258], dt.bfloat16, tag=f"y{wc}",
                                        name=f"y{wc}", bufs=3)
                            nc.vector.tensor_copy(y, psY)
                            ys.append(y)
                        pszD = psum.tile([128, 258], dt.float32, tag="pX2",
                                         name="pszD", bufs=2)
                        nc.tensor.matmul(pszD, lhsT=ys[0][:, 0:128], rhs=t_r1[0],
                                         start=True, stop=False)
                        nc.tensor.matmul(pszD, lhsT=ys[0][:, 129:257], rhs=t_r2[0],
                                         start=False, stop=False)
                        nc.tensor.matmul(pszD, lhsT=ys[1][:, 0:128], rhs=t_r1[1],
                                         start=False, stop=False)
                        nc.tensor.matmul(pszD, lhsT=ys[1][:, 129:257], rhs=t_r2[1],
                                         start=False, stop=True)
                        nc.scalar.copy(ztD[:, cl, :], pszD)
                        pszM = psum.tile([128, 258], dt.float32, tag="pX2",
                                         name="pszM", bufs=2)
                        nc.tensor.matmul(pszM, lhsT=ys[0][:, 1:129], rhs=t_r1[0],
                                         start=True, stop=False)
                        nc.tensor.matmul(pszM, lhsT=ys[0][:, 130:258], rhs=t_r2m[0],
                                         start=False, stop=False)
                        nc.tensor.matmul(pszM, lhsT=ys[1][:, 1:129], rhs=t_r1[1],
                                         start=False, stop=False)
                        nc.tensor.matmul(pszM, lhsT=ys[1][:, 130:258], rhs=t_r2m[1],
                                         start=False, stop=True)
                        nc.scalar.copy(ztM[:, cl, :], pszM)
                    nc.sync.dma_start(out=zD[b][c4:c4 + CG, :, :].transpose([1, 0, 2]),
                                      in_=ztD)
                    nc.sync.dma_start(out=zM[b][c4:c4 + CG, :, :].transpose([1, 0, 2]),
                                      in_=ztM)
                    yield

            # =================== Phase B ===================
            def emit_B(b):
                for half, (zsrc, sdst) in enumerate([(zD[b], sD[b]), (zM[b], sM[b])]):
                    nyre = pb.tile([96, 128], dt.bfloat16, tag="nyre", name="nyre", bufs=2)
                    nyim = pb.tile([96, 128], dt.bfloat16, tag="nyim", name="nyim", bufs=2)
                    for (u0, cnt) in b_groups():
                        n1 = cnt * NV
                        zg = pb.tile([96, U, 258], dt.bfloat16, tag="zg", name="zg", bufs=3)
                        nc.gpsimd.dma_start(out=zg[:, 0:cnt, :], in_=zsrc[:, u0:u0 + cnt, :])
                        zre = zg[:, 0:cnt, 0:NV]
                        zim = zg[:, 0:cnt, NV:258]
                        p1r = psum.tile([96, U, NV], dt.float32, tag="pB1", name="p1r", bufs=2)
                        p1i = psum.tile([96, U, NV], dt.float32, tag="pB2", name="p1i", bufs=2)
                        nc.tensor.matmul(p1r[:, 0:cnt, :], lhsT=t_w['w1r'], rhs=zre,
                                         start=True, stop=False)
                        nc.tensor.matmul(p1r[:, 0:cnt, :], lhsT=t_w['w1in'], rhs=zim,
                                         start=False, stop=True)
                        nc.tensor.matmul(p1i[:, 0:cnt, :], lhsT=t_w['w1i'], rhs=zre,
                                         start=True, stop=False)
                        nc.tensor.matmul(p1i[:, 0:cnt, :], lhsT=t_w['w1r'], rhs=zim,
                                         start=False, stop=True)
                        o1r = pb.tile([96, U, NV], dt.bfloat16, tag="o1r", name="o1r", bufs=3)
                        o1i = pb.tile([96, U, NV], dt.bfloat16, tag="o1i", name="o1i", bufs=3)
                        nc.scalar.activation(o1r[:, 0:cnt, :], p1r[:, 0:cnt, :],
                                             Act.Relu, bias=t_b1[:, 0:1])
                        nc.scalar.activation(o1i[:, 0:cnt, :], p1i[:, 0:cnt, :],
                                             Act.Relu, bias=t_b1[:, 1:2])
                        p2r = psum.tile([96, U, NV], dt.float32, tag="pB1", name="p2r", bufs=2)
                        p2i = psum.tile([96, U, NV], dt.float32, tag="pB2", name="p2i", bufs=2)
                        nc.tensor.matmul(p2r[:, 0:cnt, :], lhsT=t_w['w2r'], rhs=o1r[:, 0:cnt, :],
                                         start=True, stop=False)
                        nc.tensor.matmul(p2r[:, 0:cnt, :], lhsT=t_w['w2in'], rhs=o1i[:, 0:cnt, :],
                                         start=False, stop=True)
                        nc.tensor.matmul(p2i[:, 0:cnt, :], lhsT=t_w['w2i'], rhs=o1r[:, 0:cnt, :],
                                         start=True, stop=False)
                        nc.tensor.matmul(p2i[:, 0:cnt, :], lhsT=t_w['w2r'], rhs=o1i[:, 0:cnt, :],
                                         start=False, stop=True)
                        # softshrink main (v 0..127): s = relu(t+b2-lam) + min(t+b2+lam, 0)
                        sAr = pb.tile([96, U, 128], dt.bfloat16, tag="sAr", name="sAr", bufs=2)
                        sAi = pb.tile([96, U, 128], dt.bfloat16, tag="sAi", name="sAi", bufs=2)
                        sMr = pb.tile([96, U, 128], dt.bfloat16, tag="sMr", name="sMr", bufs=2)
                        sMi = pb.tile([96, U, 128], dt.bfloat16, tag="sMi", name="sMi", bufs=2)
                        str_ = pb.tile([96, U, 128], dt.bfloat16, tag="str", name="str_", bufs=2)
                        sti = pb.tile([96, U, 128], dt.bfloat16, tag="sti", name="sti", bufs=2)
                        nc.scalar.activation(sAr[:, 0:cnt, :], p2r[:, 0:cnt, 0:128],
                                             Act.Relu, bias=t_bA[:, 0:1])
                        nc.scalar.activation(sAi[:, 0:cnt, :], p2i[:, 0:cnt, 0:128],
                                             Act.Relu, bias=t_bA[:, 1:2])
                        nc.vector.tensor_scalar(sMr[:, 0:cnt, :], p2r[:, 0:cnt, 0:128],
                                                t_bM[:, 0:1], 0.0, Alu.add, Alu.min)
                        nc.vector.tensor_scalar(sMi[:, 0:cnt, :], p2i[:, 0:cnt, 0:128],
                                                t_bM[:, 1:2], 0.0, Alu.add, Alu.min)
                        nc.vector.tensor_tensor(str_[:, 0:cnt, :], sAr[:, 0:cnt, :],
                                                sMr[:, 0:cnt, :], Alu.add)
                        nc.vector.tensor_tensor(sti[:, 0:cnt, :], sAi[:, 0:cnt, :],
                                                sMi[:, 0:cnt, :], Alu.add)
                        # nyquist col (v=128): raw copy into per-half accum tiles
                        nc.scalar.activation(nyre[:, u0:u0 + cnt],
                                             p2r[:, 0:cnt, 128:129], Act.Copy)
                        nc.scalar.activation(nyim[:, u0:u0 + cnt],
                                             p2i[:, 0:cnt, 128:129], Act.Copy)
                        nc.sync.dma_start(out=sdst[:, u0:u0 + cnt, 0:128],
                                          in_=str_[:, 0:cnt, :])
                        nc.sync.dma_start(out=sdst[:, u0:u0 + cnt, 129:257],
                                          in_=sti[:, 0:cnt, :])
                        yield
                    # finish nyquist cols for this half
                    nyA = pb.tile([96, 128], dt.bfloat16, tag="nyA", name="nyA", bufs=2)
                    nyMn = pb.tile([96, 128], dt.bfloat16, tag="nyMn", name="nyMn", bufs=2)
                    nys = pb.tile([96, 128], dt.bfloat16, tag="nys", name="nys", bufs=2)
                    nysn = pb.tile([96, 128], dt.bfloat16, tag="nysn", name="nysn", bufs=2)
                    # im: +softshrink(t_im+b2i) -> col 128
                    nc.scalar.activation(nyA, nyim, Act.Relu, bias=t_bA[:, 1:2])
                    nc.vector.tensor_scalar(nyMn, nyim, t_bM[:, 1:2], 0.0, Alu.add, Alu.min)
                    nc.vector.tensor_tensor(nys, nyA, nyMn, Alu.add)
                    nc.gpsimd.dma_start(out=sdst[:, :, 128:129], in_=nys)
                    # re: -softshrink(t_re+b2r) -> col 257
                    nyA2 = pb.tile([96, 128], dt.bfloat16, tag="nyA", name="nyA2", bufs=2)
                    nyM2 = pb.tile([96, 128], dt.bfloat16, tag="nyMn", name="nyM2", bufs=2)
                    nys2 = pb.tile([96, 128], dt.bfloat16, tag="nys", name="nys2", bufs=2)
                    nc.scalar.activation(nyA2, nyre, Act.Relu, bias=t_bA[:, 0:1])
                    nc.vector.tensor_scalar(nyM2, nyre, t_bM[:, 0:1], 0.0, Alu.add, Alu.min)
                    nc.vector.tensor_tensor(nys2, nyA2, nyM2, Alu.add)
                    nc.vector.tensor_scalar_mul(nysn, nys2, -1.0)
                    nc.gpsimd.dma_start(out=sdst[:, :, 257:258], in_=nysn)
                    yield

            # =================== Phase C ===================
            def emit_C(b):
                for g in range(NG):
                    c4 = g * CG
                    stD = pc.tile([128, CG, 258], dt.bfloat16, tag="stD", name="stD", bufs=2)
                    stM = pc.tile([128, CG, 258], dt.bfloat16, tag="stM", name="stM", bufs=2)
                    nc.gpsimd.dma_start(out=stD, in_=sD[b][c4:c4 + CG, :, :].transpose([1, 0, 2]))
                    nc.gpsimd.dma_start(out=stM, in_=sM[b][c4:c4 + CG, :, :].transpose([1, 0, 2]))
                    xc = [pc.tile([128, CG, 256], dt.bfloat16, tag=f"xc{hc}",
                                  name=f"xc{hc}", bufs=2) for hc in range(2)]
                    for hc in range(2):
                        nc.gpsimd.dma_start(
                            out=xc[hc],
                            in_=xbf[b, c4:c4 + CG, hc * 128:(hc + 1) * 128, :]
                            .transpose([1, 0, 2]))
                    otw = [pc.tile([128, CG, 256], dt.bfloat16, tag=f"otw{hc}",
                                   name=f"otw{hc}", bufs=2) for hc in range(2)]
                    for cl in range(CG):
                        dd = stD[:, cl, :]
                        mm = stM[:, cl, :]
                        pP1 = psum.tile([128, 256], dt.float32, tag="pX1", name="pP1", bufs=2)
                        nc.tensor.matmul(pP1, lhsT=dd[:, 0:128], rhs=t_cr0, start=True, stop=False)
                        nc.tensor.matmul(pP1, lhsT=dd[:, 129:257], rhs=t_ni0, start=False, stop=False)
                        nc.tensor.matmul(pP1, lhsT=mm[:, 0:128], rhs=t_crM, start=False, stop=False)
                        nc.tensor.matmul(pP1, lhsT=mm[:, 129:257], rhs=t_niM, start=False, stop=True)
                        pP2 = psum.tile([128, 256], dt.float32, tag="pX1", name="pP2", bufs=2)
                        nc.tensor.matmul(pP2, lhsT=dd[:, 1:129], rhs=t_ci0, start=True, stop=False)
                        nc.tensor.matmul(pP2, lhsT=dd[:, 130:258], rhs=t_cr0, start=False, stop=False)
                        nc.tensor.matmul(pP2, lhsT=mm[:, 1:129], rhs=t_ciM, start=False, stop=False)
                        nc.tensor.matmul(pP2, lhsT=mm[:, 130:258], rhs=t_crM, start=False, stop=True)
                        p1s = pc.tile([128, 256], dt.bfloat16, tag="p1s", name="p1s", bufs=3)
                        p2s = pc.tile([128, 256], dt.bfloat16, tag="p2s", name="p2s", bufs=3)
                        nc.scalar.copy(p1s, pP1)
                        nc.scalar.copy(p2s, pP2)
                        for hc in range(2):
                            pso = psum.tile([128, 256], dt.float32, tag="pX2",
                                            name="pso", bufs=2)
                            nc.tensor.matmul(pso, lhsT=p1s[:, hc * 128:(hc + 1) * 128],
                                             rhs=t_gcx, start=True, stop=False)
                            nc.tensor.matmul(pso, lhsT=p2s[:, hc * 128:(hc + 1) * 128],
                                             rhs=t_gsx, start=False, stop=True)
                            nc.vector.tensor_tensor(otw[hc][:, cl, :], pso,
                                                    xc[hc][:, cl, :], Alu.add)
                    for hc in range(2):
                        nc.sync.dma_start(
                            out=outd[b, c4:c4 + CG, hc * 128:(hc + 1) * 128, :]
                            .transpose([1, 0, 2]),
                            in_=otw[hc])
                    yield

            # =================== zipped schedule ===================
            def run_zip(gens, ratio):
                """Round-robin with per-gen step ratios until all exhausted."""
                done = [False] * len(gens)
                while not all(done):
                    for gi, gen in enumerate(gens):
                        if done[gi]:
                            continue
                        for _ in range(ratio[gi]):
                            try:
                                next(gen)
                            except StopIteration:
                                done[gi] = True
                                break

            for _ in emit_A(0):
                pass
            run_zip([emit_B(0), emit_A(1)], [4, 1])
            run_zip([emit_C(0), emit_B(1)], [1, 4])
            for _ in emit_C(1):
                pass

    nc.compile()
    return nc


_NC_CACHE = {}


def _get_nc():
    if 'nc' not in _NC_CACHE:
        _NC_CACHE['nc'] = build_nc()
    return _NC_CACHE['nc']


def make_in_maps(x, w1, b1, w2, b2):
    hc = make_host_consts()
    x = np.asarray(x, dtype=np.float32)
    in_maps = []
    for k in range(NCORES):
        xk = np.ascontiguousarray(x[:, BLK * k:BLK * (k + 1)]).astype(BF16)
        b1k = b1[k, :, 0, 0, :].astype(np.float32)
        b2k = b2[k, :, 0, 0, :].astype(np.float32)
        m = dict(
            xbf=xk,
            chh=hc['chh'], r1=hc['r1'], r2=hc['r2'], r2m=hc['r2m'],
            cr0=hc['cr0'], ci0=hc['ci0'], ni0=hc['ni0'],
            crM=hc['crM'], ciM=hc['ciM'], niM=hc['niM'],
            gcx=hc['gcx'], gsx=hc['gsx'],
            w1r=w1[k, :, :, 0].astype(BF16),
            w1i=w1[k, :, :, 1].astype(BF16),
            w1in=(-w1[k, :, :, 1]).astype(BF16),
            w2r=w2[k, :, :, 0].astype(BF16),
            w2i=w2[k, :, :, 1].astype(BF16),
            w2in=(-w2[k, :, :, 1]).astype(BF16),
            b1c=np.ascontiguousarray(b1k),
            bAc=np.ascontiguousarray(b2k - LAM),
            bMc=np.ascontiguousarray(b2k + LAM),
        )
        in_maps.append(m)
    return in_maps


def kernel(x, w1, b1, w2, b2):
    from concourse.bass_utils import run_bass_kernel_spmd
    nc = _get_nc()
    in_maps = make_in_maps(np.asarray(x), np.asarray(w1), np.asarray(b1),
                           np.asarray(w2), np.asarray(b2))
    res = run_bass_kernel_spmd(nc, in_maps, core_ids=list(range(NCORES)))
    outs = [res.results[k]['out'].astype(np.float32) for k in range(NCORES)]
    return np.concatenate(outs, axis=1)


# revision 8
# speedup vs baseline: 1.3048x; 1.1029x over previous
"""Trainium2 Bass kernel for DistributedAFNO2D (v2).

Problem: x(2,768,256,256) f32; per-block (8 blocks of 96 ch) spectral MLP:
  out = irfft2( softshrink( W2*relu(W1*rfft2(x) + b1) + b2 ) ) + x
Sharding: block k -> core k (8 cores). No collectives.

v2 design vs v1 (1.88ms):
 - Phase A (fwd FFT): S1 computes only u=0..128 (rfft symmetry), S2 derives
   the mirror half u=255..128 from conj(Y) with sign-flipped EW consts.
   Mirror rows live in their own DRAM buffer (zbufM, rows j <-> u=255-j);
   all reversals are absorbed into host-built constant row orders.
 - Phase B (spectral MLP): weight-stationary matmuls (96-col LDW hides under
   N=387 moving), 3 u-rows per group, re/im in separate PSUM tiles.
   softshrink = relu(t+b2-lam) + min(t+b2+lam, 0) via ACT+DVE.
 - Phase C (inv FFT): DC(v=0) folded into the main matmuls (P1 rows=Qr[0..127]
   with Gc[0]=1/16 row); Nyquist v=128 folded via P2 row127=-Qr[128] paired
   with -Gc[128] in GSX. No tiny DC matmuls, no scalar_tensor_tensor.
 - Per-batch DRAM tensors + zipped emission (A0 | B0+A1 | C0+B1 | C1) keep
   the PE warm; x stays resident in SBUF for the residual add.
 - Output bf16, upcast on host.

sd column layout (C1 lhsT windows, per u-row):
  [ s_re v0..127 | s_im v128 | s_im v0..127 | -s_re v128 ]
   P1-A=0:128 (x)CHIr ; P1-B=129:257 (x)-CHIi -> rows Qr v0..127
   P2-A=1:129 (x)CHIi ; P2-B=130:258 (x)CHIr  -> rows Qi v1..127, row127=-Qr[128]
"""
import sys
import numpy as np

sys.path.insert(0, "/opt/trn_rl_repo")

import ml_dtypes

BF16 = ml_dtypes.bfloat16

H = 256
W = 256
NV = 129
BLK = 96
NCORES = 8
B = 2
LAM = 0.01
U = 3  # u-rows per phase-B group
CG = 4  # channels per DMA batch


def b_groups():
    """(u0, cnt) groups covering 128 rows."""
    out = []
    u0 = 0
    while u0 < 128:
        cnt = min(U, 128 - u0)
        out.append((u0, cnt))
        u0 += cnt
    return out


def make_host_consts():
    I = np.eye(H, dtype=np.float64)
    F = np.fft.fft(I, axis=0, norm='ortho')       # F[u,h]
    Fi = np.fft.ifft(I, axis=0, norm='ortho')     # Fi[h,u]
    CHr = F.real.T.copy()                          # [h,u]
    CHi = F.imag.T.copy()
    EWr = F.real.T[:, :NV].copy()                  # [w,v]
    EWi = F.imag.T[:, :NV].copy()
    CHIr = Fi.real.T.copy()                        # [u,h]
    CHIi = Fi.imag.T.copy()
    Ir = np.eye(NV)
    Gc = np.fft.irfft(Ir, n=W, axis=-1, norm='ortho')        # [v,w]
    Gs = np.fft.irfft(1j * Ir, n=W, axis=-1, norm='ortho')   # [v,w]

    c = {}
    # A1: [h-chunk, (CHr u0..128 | CHi u0..128)]
    c['chh'] = np.stack([
        np.concatenate([CHr[j * 128:(j + 1) * 128, 0:NV],
                        CHi[j * 128:(j + 1) * 128, 0:NV]], axis=1)
        for j in range(2)])
    # A2 direct: R1 = [EWr|EWi], R2 = [-EWi|EWr]; mirror: R2m = [EWi|-EWr]
    c['r1'] = np.stack([
        np.concatenate([EWr[j * 128:(j + 1) * 128], EWi[j * 128:(j + 1) * 128]], axis=1)
        for j in range(2)])
    c['r2'] = np.stack([
        np.concatenate([-EWi[j * 128:(j + 1) * 128], EWr[j * 128:(j + 1) * 128]], axis=1)
        for j in range(2)])
    c['r2m'] = (-c['r2']).copy()
    # C1 rhs consts: direct rows u=0..127; mirror rows j <-> u=255-j
    perm = np.array([255 - j for j in range(128)])
    c['cr0'] = CHIr[0:128]
    c['ci0'] = CHIi[0:128]
    c['ni0'] = -CHIi[0:128]
    c['crM'] = CHIr[perm]
    c['ciM'] = CHIi[perm]
    c['niM'] = -CHIi[perm]
    # C2: GCX rows v=0..127 (incl DC); GSX rows = [Gs v1..127 ; -Gc v128]
    c['gcx'] = Gc[0:128]
    c['gsx'] = np.concatenate([Gs[1:128], -Gc[128:129]], axis=0)
    return {k: v.astype(BF16) for k, v in c.items()}


def build_nc():
    import concourse.bass as bass
    import concourse.tile as tile
    from concourse import bacc, mybir

    dt = mybir.dt
    Alu = mybir.AluOpType
    Act = mybir.ActivationFunctionType
    nc = bacc.Bacc("TRN2", target_bir_lowering=False, debug=False)

    # ---- I/O ----
    xbf = nc.dram_tensor("xbf", [B, BLK, H, W], dt.bfloat16, kind="ExternalInput").ap()
    chh = nc.dram_tensor("chh", [2, 128, 258], dt.bfloat16, kind="ExternalInput").ap()
    r1 = nc.dram_tensor("r1", [2, 128, 258], dt.bfloat16, kind="ExternalInput").ap()
    r2 = nc.dram_tensor("r2", [2, 128, 258], dt.bfloat16, kind="ExternalInput").ap()
    r2m = nc.dram_tensor("r2m", [2, 128, 258], dt.bfloat16, kind="ExternalInput").ap()
    cr0 = nc.dram_tensor("cr0", [128, 256], dt.bfloat16, kind="ExternalInput").ap()
    ci0 = nc.dram_tensor("ci0", [128, 256], dt.bfloat16, kind="ExternalInput").ap()
    ni0 = nc.dram_tensor("ni0", [128, 256], dt.bfloat16, kind="ExternalInput").ap()
    crM = nc.dram_tensor("crM", [128, 256], dt.bfloat16, kind="ExternalInput").ap()
    ciM = nc.dram_tensor("ciM", [128, 256], dt.bfloat16, kind="ExternalInput").ap()
    niM = nc.dram_tensor("niM", [128, 256], dt.bfloat16, kind="ExternalInput").ap()
    gcx = nc.dram_tensor("gcx", [128, 256], dt.bfloat16, kind="ExternalInput").ap()
    gsx = nc.dram_tensor("gsx", [128, 256], dt.bfloat16, kind="ExternalInput").ap()
    wts = {n: nc.dram_tensor(n, [96, 96], dt.bfloat16, kind="ExternalInput").ap()
           for n in ['w1r', 'w1i', 'w1in', 'w2r', 'w2i', 'w2in']}
    b1c = nc.dram_tensor("b1c", [96, 2], dt.float32, kind="ExternalInput").ap()
    bAc = nc.dram_tensor("bAc", [96, 2], dt.float32, kind="ExternalInput").ap()  # b2-lam
    bMc = nc.dram_tensor("bMc", [96, 2], dt.float32, kind="ExternalInput").ap()  # b2+lam
    outd = nc.dram_tensor("out", [B, BLK, H, W], dt.bfloat16, kind="ExternalOutput").ap()

    # DRAM scratch, separate tensors per batch to avoid cross-batch false deps
    zD = [nc.dram_tensor(f"zD{b}", [BLK, 128, 258], dt.bfloat16).ap() for b in range(B)]
    zM = [nc.dram_tensor(f"zM{b}", [BLK, 128, 258], dt.bfloat16).ap() for b in range(B)]
    sD = [nc.dram_tensor(f"sD{b}", [BLK, 128, 258], dt.bfloat16).ap() for b in range(B)]
    sM = [nc.dram_tensor(f"sM{b}", [BLK, 128, 258], dt.bfloat16).ap() for b in range(B)]

    NG = BLK // CG  # 24 channel groups

    with tile.TileContext(nc) as tc:
        from contextlib import ExitStack
        with ExitStack() as ctx:
            consts = ctx.enter_context(tc.tile_pool(name="consts", bufs=1))
            xres = ctx.enter_context(tc.tile_pool(name="xres", bufs=1))
            pa = ctx.enter_context(tc.tile_pool(name="pa", bufs=3))
            pb = ctx.enter_context(tc.tile_pool(name="pb", bufs=3))
            pc = ctx.enter_context(tc.tile_pool(name="pc", bufs=3))
            psum = ctx.enter_context(tc.tile_pool(name="psum", bufs=2, space="PSUM"))

            # ---- constants ----
            def ld(name, ap_, shape):
                t = consts.tile(shape, dt.bfloat16, tag=name, name=name)
                nc.sync.dma_start(out=t, in_=ap_)
                return t

            t_chh = [ld(f"chh{j}", chh[j], [128, 258]) for j in range(2)]
            t_r1 = [ld(f"r1{j}", r1[j], [128, 258]) for j in range(2)]
            t_r2 = [ld(f"r2{j}", r2[j], [128, 258]) for j in range(2)]
            t_r2m = [ld(f"r2m{j}", r2m[j], [128, 258]) for j in range(2)]
            t_cr0 = ld("cr0", cr0, [128, 256])
            t_ci0 = ld("ci0", ci0, [128, 256])
            t_ni0 = ld("ni0", ni0, [128, 256])
            t_crM = ld("crM", crM, [128, 256])
            t_ciM = ld("ciM", ciM, [128, 256])
            t_niM = ld("niM", niM, [128, 256])
            t_gcx = ld("gcx", gcx, [128, 256])
            t_gsx = ld("gsx", gsx, [128, 256])
            t_w = {n: ld(n, ap_, [96, 96]) for n, ap_ in wts.items()}
            t_b1 = consts.tile([96, 2], dt.float32, tag="b1", name="t_b1")
            nc.sync.dma_start(out=t_b1, in_=b1c)
            t_bA = consts.tile([96, 2], dt.float32, tag="bA", name="t_bA")
            nc.sync.dma_start(out=t_bA, in_=bAc)
            t_bM = consts.tile([96, 2], dt.float32, tag="bM", name="t_bM")
            nc.sync.dma_start(out=t_bM, in_=bMc)

            # x resident tiles, one per (channel-group, h-chunk), reused across batches
            xr = [[xres.tile([128, CG, 256], dt.bfloat16, tag=f"xr{g}_{hc}",
                             name=f"xr{g}_{hc}") for hc in range(2)]
                  for g in range(NG)]

            # =================== Phase A ===================
            def emit_A(b):
                for g in range(NG):
                    c4 = g * CG
                    for hc in range(2):
                        nc.sync.dma_start(
                            out=xr[g][hc],
                            in_=xbf[b, c4:c4 + CG, hc * 128:(hc + 1) * 128, :]
                            .transpose([1, 0, 2]))
                    ztD = pa.tile([128, CG, 258], dt.bfloat16, tag="ztD", name="ztD", bufs=2)
                    ztM = pa.tile([128, CG, 258], dt.bfloat16, tag="ztM", name="ztM", bufs=2)
                    for cl in range(CG):
                        ys = []
                        for wc in range(2):
                            psY = psum.tile([128, 258], dt.float32, tag="pX1",
                                            name="psY", bufs=2)
                            for hc in range(2):
                                nc.tensor.matmul(
                                    psY, lhsT=xr[g][hc][:, cl, wc * 128:(wc + 1) * 128],
                                    rhs=t_chh[hc], start=(hc == 0), stop=(hc == 1))
                            y = pa.tile([128, 258], dt.bfloat16, tag=f"y{wc}",
                                        name=f"y{wc}", bufs=3)
                            nc.vector.tensor_copy(y, psY)
                            ys.append(y)
                        pszD = psum.tile([128, 258], dt.float32, tag="pX2",
                                         name="pszD", bufs=2)
                        nc.tensor.matmul(pszD, lhsT=ys[0][:, 0:128], rhs=t_r1[0],
                                         start=True, stop=False)
                        nc.tensor.matmul(pszD, lhsT=ys[0][:, 129:257], rhs=t_r2[0],
                                         start=False, stop=False)
                        nc.tensor.matmul(pszD, lhsT=ys[1][:, 0:128], rhs=t_r1[1],
                                         start=False, stop=False)
                        nc.tensor.matmul(pszD, lhsT=ys[1][:, 129:257], rhs=t_r2[1],
                                         start=False, stop=True)
                        nc.scalar.copy(ztD[:, cl, :], pszD)
                        pszM = psum.tile([128, 258], dt.float32, tag="pX2",
                                         name="pszM", bufs=2)
                        nc.tensor.matmul(pszM, lhsT=ys[0][:, 1:129], rhs=t_r1[0],
                                         start=True, stop=False)
                        nc.tensor.matmul(pszM, lhsT=ys[0][:, 130:258], rhs=t_r2m[0],
                                         start=False, stop=False)
                        nc.tensor.matmul(pszM, lhsT=ys[1][:, 1:129], rhs=t_r1[1],
                                         start=False, stop=False)
                        nc.tensor.matmul(pszM, lhsT=ys[1][:, 130:258], rhs=t_r2m[1],
                                         start=False, stop=True)
                        nc.scalar.copy(ztM[:, cl, :], pszM)
                    nc.gpsimd.dma_start(out=zD[b][c4:c4 + CG, :, :].transpose([1, 0, 2]),
                                        in_=ztD)
                    nc.gpsimd.dma_start(out=zM[b][c4:c4 + CG, :, :].transpose([1, 0, 2]),
                                        in_=ztM)
                    yield

            # =================== Phase B ===================
            def emit_B(b):
                for half, (zsrc, sdst) in enumerate([(zD[b], sD[b]), (zM[b], sM[b])]):
                    for (u0, cnt) in b_groups():
                        zg = pb.tile([96, U, 258], dt.bfloat16, tag="zg", name="zg", bufs=3)
                        nc.sync.dma_start(out=zg[:, 0:cnt, :], in_=zsrc[:, u0:u0 + cnt, :])
                        zre = zg[:, 0:cnt, 0:NV]
                        zim = zg[:, 0:cnt, NV:258]
                        # psum padded to 130 inner cols (even) for ACT/DVE perf modes
                        p1r = psum.tile([96, U, 130], dt.float32, tag="pB1", name="p1r", bufs=2)
                        p1i = psum.tile([96, U, 130], dt.float32, tag="pB2", name="p1i", bufs=2)
                        nc.tensor.matmul(p1r[:, 0:cnt, 0:NV], lhsT=t_w['w1r'], rhs=zre,
                                         start=True, stop=False)
                        nc.tensor.matmul(p1r[:, 0:cnt, 0:NV], lhsT=t_w['w1in'], rhs=zim,
                                         start=False, stop=True)
                        nc.tensor.matmul(p1i[:, 0:cnt, 0:NV], lhsT=t_w['w1i'], rhs=zre,
                                         start=True, stop=False)
                        nc.tensor.matmul(p1i[:, 0:cnt, 0:NV], lhsT=t_w['w1r'], rhs=zim,
                                         start=False, stop=True)
                        o1r = pb.tile([96, U, 130], dt.bfloat16, tag="o1r", name="o1r", bufs=3)
                        o1i = pb.tile([96, U, 130], dt.bfloat16, tag="o1i", name="o1i", bufs=3)
                        nc.scalar.activation(o1r[:, 0:cnt, 0:NV], p1r[:, 0:cnt, 0:NV],
                                             Act.Relu, bias=t_b1[:, 0:1])
                        nc.scalar.activation(o1i[:, 0:cnt, 0:NV], p1i[:, 0:cnt, 0:NV],
                                             Act.Relu, bias=t_b1[:, 1:2])
                        p2r = psum.tile([96, U, 130], dt.float32, tag="pB1", name="p2r", bufs=2)
                        p2i = psum.tile([96, U, 130], dt.float32, tag="pB2", name="p2i", bufs=2)
                        nc.tensor.matmul(p2r[:, 0:cnt, 0:NV], lhsT=t_w['w2r'],
                                         rhs=o1r[:, 0:cnt, 0:NV], start=True, stop=False)
                        nc.tensor.matmul(p2r[:, 0:cnt, 0:NV], lhsT=t_w['w2in'],
                                         rhs=o1i[:, 0:cnt, 0:NV], start=False, stop=True)
                        nc.tensor.matmul(p2i[:, 0:cnt, 0:NV], lhsT=t_w['w2i'],
                                         rhs=o1r[:, 0:cnt, 0:NV], start=True, stop=False)
                        nc.tensor.matmul(p2i[:, 0:cnt, 0:NV], lhsT=t_w['w2r'],
                                         rhs=o1i[:, 0:cnt, 0:NV], start=False, stop=True)
                        # softshrink: s = relu(t+b2-lam) + min(t+b2+lam, 0)
                        # str_: re main (v0..127) -> sd cols 0:128
                        # st2:  [s_im v128 | s_im v0..127 | -s_re v128] -> sd cols 128:258
                        sAr = pb.tile([96, U, 128], dt.bfloat16, tag="sAr", name="sAr", bufs=2)
                        sAi = pb.tile([96, U, 128], dt.bfloat16, tag="sAi", name="sAi", bufs=2)
                        sMr = pb.tile([96, U, 128], dt.bfloat16, tag="sMr", name="sMr", bufs=2)
                        sMi = pb.tile([96, U, 128], dt.bfloat16, tag="sMi", name="sMi", bufs=2)
                        str_ = pb.tile([96, U, 128], dt.bfloat16, tag="str", name="str_", bufs=2)
                        st2 = pb.tile([96, U, 130], dt.bfloat16, tag="st2", name="st2", bufs=2)
                        nc.scalar.activation(sAr[:, 0:cnt, :], p2r[:, 0:cnt, 0:128],
                                             Act.Relu, bias=t_bA[:, 0:1])
                        nc.scalar.activation(sAi[:, 0:cnt, :], p2i[:, 0:cnt, 0:128],
                                             Act.Relu, bias=t_bA[:, 1:2])
                        nc.vector.tensor_scalar(sMr[:, 0:cnt, :], p2r[:, 0:cnt, 0:128],
                                                t_bM[:, 0:1], 0.0, Alu.add, Alu.min)
                        nc.vector.tensor_scalar(sMi[:, 0:cnt, :], p2i[:, 0:cnt, 0:128],
                                                t_bM[:, 1:2], 0.0, Alu.add, Alu.min)
                        nc.vector.tensor_tensor(str_[:, 0:cnt, :], sAr[:, 0:cnt, :],
                                                sMr[:, 0:cnt, :], Alu.add)
                        nc.vector.tensor_tensor(st2[:, 0:cnt, 1:129], sAi[:, 0:cnt, :],
                                                sMi[:, 0:cnt, :], Alu.add)
                        # nyquist col v=128: im (+) -> st2 col 0 ; re (-) -> st2 col 129
                        nyA = pb.tile([96, U, 2], dt.bfloat16, tag="nyA", name="nyA", bufs=2)
                        nyM = pb.tile([96, U, 2], dt.bfloat16, tag="nyM", name="nyM", bufs=2)
                        nys = pb.tile([96, U, 2], dt.bfloat16, tag="nys", name="nys", bufs=2)
                        nc.scalar.activation(nyA[:, 0:cnt, 0:1], p2i[:, 0:cnt, 128:129],
                                             Act.Relu, bias=t_bA[:, 1:2])
                        nc.scalar.activation(nyA[:, 0:cnt, 1:2], p2r[:, 0:cnt, 128:129],
                                             Act.Relu, bias=t_bA[:, 0:1])
                        nc.vector.tensor_scalar(nyM[:, 0:cnt, 0:1], p2i[:, 0:cnt, 128:129],
                                                t_bM[:, 1:2], 0.0, Alu.add, Alu.min)
                        nc.vector.tensor_scalar(nyM[:, 0:cnt, 1:2], p2r[:, 0:cnt, 128:129],
                                                t_bM[:, 0:1], 0.0, Alu.add, Alu.min)
                        nc.vector.tensor_tensor(nys[:, 0:cnt, :], nyA[:, 0:cnt, :],
                                                nyM[:, 0:cnt, :], Alu.add)
                        nc.vector.tensor_copy(st2[:, 0:cnt, 0:1], nys[:, 0:cnt, 0:1])
                        nc.vector.tensor_scalar_mul(st2[:, 0:cnt, 129:130], nys[:, 0:cnt, 1:2],
                                                    -1.0)
                        nc.gpsimd.dma_start(out=sdst[:, u0:u0 + cnt, 0:128],
                                            in_=str_[:, 0:cnt, :])
                        nc.gpsimd.dma_start(out=sdst[:, u0:u0 + cnt, 128:258],
                                            in_=st2[:, 0:cnt, :])
                        yield

            # =================== Phase C ===================
            def emit_C(b):
                for g in range(NG):
                    c4 = g * CG
                    stD = pc.tile([128, CG, 258], dt.bfloat16, tag="stD", name="stD", bufs=2)
                    stM = pc.tile([128, CG, 258], dt.bfloat16, tag="stM", name="stM", bufs=2)
                    nc.sync.dma_start(out=stD, in_=sD[b][c4:c4 + CG, :, :].transpose([1, 0, 2]))
                    nc.sync.dma_start(out=stM, in_=sM[b][c4:c4 + CG, :, :].transpose([1, 0, 2]))
                    xc = [pc.tile([128, CG, 256], dt.bfloat16, tag=f"xc{hc}",
                                  name=f"xc{hc}", bufs=2) for hc in range(2)]
                    for hc in range(2):
                        nc.sync.dma_start(
                            out=xc[hc],
                            in_=xbf[b, c4:c4 + CG, hc * 128:(hc + 1) * 128, :]
                            .transpose([1, 0, 2]))
                    otw = [pc.tile([128, CG, 256], dt.bfloat16, tag=f"otw{hc}",
                                   name=f"otw{hc}", bufs=2) for hc in range(2)]
                    for cl in range(CG):
                        dd = stD[:, cl, :]
                        mm = stM[:, cl, :]
                        pP1 = psum.tile([128, 256], dt.float32, tag="pX1", name="pP1", bufs=2)
                        nc.tensor.matmul(pP1, lhsT=dd[:, 0:128], rhs=t_cr0, start=True, stop=False)
                        nc.tensor.matmul(pP1, lhsT=dd[:, 129:257], rhs=t_ni0, start=False, stop=False)
                        nc.tensor.matmul(pP1, lhsT=mm[:, 0:128], rhs=t_crM, start=False, stop=False)
                        nc.tensor.matmul(pP1, lhsT=mm[:, 129:257], rhs=t_niM, start=False, stop=True)
                        pP2 = psum.tile([128, 256], dt.float32, tag="pX1", name="pP2", bufs=2)
                        nc.tensor.matmul(pP2, lhsT=dd[:, 1:129], rhs=t_ci0, start=True, stop=False)
                        nc.tensor.matmul(pP2, lhsT=dd[:, 130:258], rhs=t_cr0, start=False, stop=False)
                        nc.tensor.matmul(pP2, lhsT=mm[:, 1:129], rhs=t_ciM, start=False, stop=False)
                        nc.tensor.matmul(pP2, lhsT=mm[:, 130:258], rhs=t_crM, start=False, stop=True)
                        p1s = pc.tile([128, 256], dt.bfloat16, tag="p1s", name="p1s", bufs=3)
                        p2s = pc.tile([128, 256], dt.bfloat16, tag="p2s", name="p2s", bufs=3)
                        nc.scalar.copy(p1s, pP1)
                        nc.scalar.copy(p2s, pP2)
                        for hc in range(2):
                            pso = psum.tile([128, 256], dt.float32, tag="pX2",
                                            name="pso", bufs=2)
                            nc.tensor.matmul(pso, lhsT=p1s[:, hc * 128:(hc + 1) * 128],
                                             rhs=t_gcx, start=True, stop=False)
                            nc.tensor.matmul(pso, lhsT=p2s[:, hc * 128:(hc + 1) * 128],
                                             rhs=t_gsx, start=False, stop=True)
                            nc.vector.tensor_tensor(otw[hc][:, cl, :], pso,
                                                    xc[hc][:, cl, :], Alu.add)
                    for hc in range(2):
                        nc.gpsimd.dma_start(
                            out=outd[b, c4:c4 + CG, hc * 128:(hc + 1) * 128, :]
                            .transpose([1, 0, 2]),
                            in_=otw[hc])
                    yield

            # =================== zipped schedule ===================
            def run_zip(gens, ratio):
                """Round-robin with per-gen step ratios until all exhausted."""
                done = [False] * len(gens)
                while not all(done):
                    for gi, gen in enumerate(gens):
                        if done[gi]:
                            continue
                        for _ in range(ratio[gi]):
                            try:
                                next(gen)
                            except StopIteration:
                                done[gi] = True
                                break

            for _ in emit_A(0):
                pass
            run_zip([emit_B(0), emit_A(1)], [4, 1])
            run_zip([emit_C(0), emit_B(1)], [1, 4])
            for _ in emit_C(1):
                pass

    nc.compile()
    return nc


_NC_CACHE = {}


def _get_nc():
    if 'nc' not in _NC_CACHE:
        _NC_CACHE['nc'] = build_nc()
    return _NC_CACHE['nc']


def make_in_maps(x, w1, b1, w2, b2):
    hc = make_host_consts()
    x = np.asarray(x, dtype=np.float32)
    in_maps = []
    for k in range(NCORES):
        xk = np.ascontiguousarray(x[:, BLK * k:BLK * (k + 1)]).astype(BF16)
        b1k = b1[k, :, 0, 0, :].astype(np.float32)
        b2k = b2[k, :, 0, 0, :].astype(np.float32)
        m = dict(
            xbf=xk,
            chh=hc['chh'], r1=hc['r1'], r2=hc['r2'], r2m=hc['r2m'],
            cr0=hc['cr0'], ci0=hc['ci0'], ni0=hc['ni0'],
            crM=hc['crM'], ciM=hc['ciM'], niM=hc['niM'],
            gcx=hc['gcx'], gsx=hc['gsx'],
            w1r=w1[k, :, :, 0].astype(BF16),
            w1i=w1[k, :, :, 1].astype(BF16),
            w1in=(-w1[k, :, :, 1]).astype(BF16),
            w2r=w2[k, :, :, 0].astype(BF16),
            w2i=w2[k, :, :, 1].astype(BF16),
            w2in=(-w2[k, :, :, 1]).astype(BF16),
            b1c=np.ascontiguousarray(b1k),
            bAc=np.ascontiguousarray(b2k - LAM),
            bMc=np.ascontiguousarray(b2k + LAM),
        )
        in_maps.append(m)
    return in_maps


def kernel(x, w1, b1, w2, b2):
    from concourse.bass_utils import run_bass_kernel_spmd
    nc = _get_nc()
    in_maps = make_in_maps(np.asarray(x), np.asarray(w1), np.asarray(b1),
                           np.asarray(w2), np.asarray(b2))
    res = run_bass_kernel_spmd(nc, in_maps, core_ids=list(range(NCORES)))
    outs = [res.results[k]['out'].astype(np.float32) for k in range(NCORES)]
    return np.concatenate(outs, axis=1)


# revision 18
# speedup vs baseline: 1.6786x; 1.2864x over previous
"""Trainium2 Bass kernel for DistributedAFNO2D (v2).

Problem: x(2,768,256,256) f32; per-block (8 blocks of 96 ch) spectral MLP:
  out = irfft2( softshrink( W2*relu(W1*rfft2(x) + b1) + b2 ) ) + x
Sharding: block k -> core k (8 cores). No collectives.

v2 design vs v1 (1.88ms):
 - Phase A (fwd FFT): S1 computes only u=0..128 (rfft symmetry), S2 derives
   the mirror half u=255..128 from conj(Y) with sign-flipped EW consts.
   Mirror rows live in their own DRAM buffer (zbufM, rows j <-> u=255-j);
   all reversals are absorbed into host-built constant row orders.
 - Phase B (spectral MLP): weight-stationary matmuls (96-col LDW hides under
   N=387 moving), 3 u-rows per group, re/im in separate PSUM tiles.
   softshrink = relu(t+b2-lam) + min(t+b2+lam, 0) via ACT+DVE.
 - Phase C (inv FFT): DC(v=0) folded into the main matmuls (P1 rows=Qr[0..127]
   with Gc[0]=1/16 row); Nyquist v=128 folded via P2 row127=-Qr[128] paired
   with -Gc[128] in GSX. No tiny DC matmuls, no scalar_tensor_tensor.
 - Per-batch DRAM tensors + zipped emission (A0 | B0+A1 | C0+B1 | C1) keep
   the PE warm; x stays resident in SBUF for the residual add.
 - Output bf16, upcast on host.

sd column layout (C1 lhsT windows, per u-row):
  [ s_re v0..127 | s_im v128 | s_im v0..127 | -s_re v128 ]
   P1-A=0:128 (x)CHIr ; P1-B=129:257 (x)-CHIi -> rows Qr v0..127
   P2-A=1:129 (x)CHIi ; P2-B=130:258 (x)CHIr  -> rows Qi v1..127, row127=-Qr[128]
"""
import sys
import numpy as np

sys.path.insert(0, "/opt/trn_rl_repo")

import ml_dtypes

BF16 = ml_dtypes.bfloat16

H = 256
W = 256
NV = 129
BLK = 96
NCORES = 8
B = 2
LAM = 0.01
U = 3  # u-rows per phase-B group
CG = 4  # channels per DMA batch


def b_groups():
    """(u0, cnt) groups covering 128 rows."""
    out = []
    u0 = 0
    while u0 < 128:
        cnt = min(U, 128 - u0)
        out.append((u0, cnt))
        u0 += cnt
    return out


def make_host_consts():
    I = np.eye(H, dtype=np.float64)
    F = np.fft.fft(I, axis=0, norm='ortho')       # F[u,h]
    Fi = np.fft.ifft(I, axis=0, norm='ortho')     # Fi[h,u]
    CHr = F.real.T.copy()                          # [h,u]
    CHi = F.imag.T.copy()
    EWr = F.real.T[:, :NV].copy()                  # [w,v]
    EWi = F.imag.T[:, :NV].copy()
    CHIr = Fi.real.T.copy()                        # [u,h]
    CHIi = Fi.imag.T.copy()
    Ir = np.eye(NV)
    Gc = np.fft.irfft(Ir, n=W, axis=-1, norm='ortho')        # [v,w]
    Gs = np.fft.irfft(1j * Ir, n=W, axis=-1, norm='ortho')   # [v,w]

    c = {}
    # A1: [h-chunk, (CHr u0..128 | CHi u0..128)]
    c['chh'] = np.stack([
        np.concatenate([CHr[j * 128:(j + 1) * 128, 0:NV],
                        CHi[j * 128:(j + 1) * 128, 0:NV]], axis=1)
        for j in range(2)])
    # A2 direct: R1 = [EWr|EWi], R2 = [-EWi|EWr]; mirror: R2m = [EWi|-EWr]
    c['r1'] = np.stack([
        np.concatenate([EWr[j * 128:(j + 1) * 128], EWi[j * 128:(j + 1) * 128]], axis=1)
        for j in range(2)])
    c['r2'] = np.stack([
        np.concatenate([-EWi[j * 128:(j + 1) * 128], EWr[j * 128:(j + 1) * 128]], axis=1)
        for j in range(2)])
    c['r2m'] = (-c['r2']).copy()
    # C1 rhs consts: direct rows u=0..127; mirror rows j <-> u=255-j
    perm = np.array([255 - j for j in range(128)])
    c['cr0'] = CHIr[0:128]
    c['ci0'] = CHIi[0:128]
    c['ni0'] = -CHIi[0:128]
    c['crM'] = CHIr[perm]
    c['ciM'] = CHIi[perm]
    c['niM'] = -CHIi[perm]
    # C2: GCX rows v=0..127 (incl DC); GSX rows = [Gs v1..127 ; -Gc v128]
    c['gcx'] = Gc[0:128]
    c['gsx'] = np.concatenate([Gs[1:128], -Gc[128:129]], axis=0)
    return {k: v.astype(BF16) for k, v in c.items()}


def build_nc():
    import concourse.bass as bass
    import concourse.tile as tile
    from concourse import bacc, mybir

    dt = mybir.dt
    Alu = mybir.AluOpType
    Act = mybir.ActivationFunctionType
    nc = bacc.Bacc("TRN2", target_bir_lowering=False, debug=False)

    # ---- I/O ----
    xbf = nc.dram_tensor("xbf", [B, BLK, H, W], dt.bfloat16, kind="ExternalInput").ap()
    chh = nc.dram_tensor("chh", [2, 128, 258], dt.bfloat16, kind="ExternalInput").ap()
    r1 = nc.dram_tensor("r1", [2, 128, 258], dt.bfloat16, kind="ExternalInput").ap()
    r2 = nc.dram_tensor("r2", [2, 128, 258], dt.bfloat16, kind="ExternalInput").ap()
    r2m = nc.dram_tensor("r2m", [2, 128, 258], dt.bfloat16, kind="ExternalInput").ap()
    cr0 = nc.dram_tensor("cr0", [128, 256], dt.bfloat16, kind="ExternalInput").ap()
    ci0 = nc.dram_tensor("ci0", [128, 256], dt.bfloat16, kind="ExternalInput").ap()
    ni0 = nc.dram_tensor("ni0", [128, 256], dt.bfloat16, kind="ExternalInput").ap()
    crM = nc.dram_tensor("crM", [128, 256], dt.bfloat16, kind="ExternalInput").ap()
    ciM = nc.dram_tensor("ciM", [128, 256], dt.bfloat16, kind="ExternalInput").ap()
    niM = nc.dram_tensor("niM", [128, 256], dt.bfloat16, kind="ExternalInput").ap()
    gcx = nc.dram_tensor("gcx", [128, 256], dt.bfloat16, kind="ExternalInput").ap()
    gsx = nc.dram_tensor("gsx", [128, 256], dt.bfloat16, kind="ExternalInput").ap()
    wts = {n: nc.dram_tensor(n, [96, 96], dt.bfloat16, kind="ExternalInput").ap()
           for n in ['w1r', 'w1i', 'w1in', 'w2r', 'w2i', 'w2in']}
    b1c = nc.dram_tensor("b1c", [96, 2], dt.float32, kind="ExternalInput").ap()
    bAc = nc.dram_tensor("bAc", [96, 2], dt.float32, kind="ExternalInput").ap()  # b2-lam
    bMc = nc.dram_tensor("bMc", [96, 2], dt.float32, kind="ExternalInput").ap()  # b2+lam
    outd = nc.dram_tensor("out", [B, BLK, H, W], dt.bfloat16, kind="ExternalOutput").ap()

    # DRAM scratch, separate tensors per batch to avoid cross-batch false deps
    # sd row layout (260 cols): [s_re v0..127 | s_im v128 | pad | s_im v0..127 |
    #                            -s_re v128 | pad]
    # C1 windows: P1-A=0:128 P2-A=1:129 P1-B=130:258 P2-B=131:259
    zD = [nc.dram_tensor(f"zD{b}", [BLK, 128, 258], dt.bfloat16).ap() for b in range(B)]
    zM = [nc.dram_tensor(f"zM{b}", [BLK, 128, 258], dt.bfloat16).ap() for b in range(B)]
    sD = [nc.dram_tensor(f"sD{b}", [BLK, 128, 260], dt.bfloat16).ap() for b in range(B)]
    sM = [nc.dram_tensor(f"sM{b}", [BLK, 128, 260], dt.bfloat16).ap() for b in range(B)]

    NG = BLK // CG  # 24 channel groups

    with tile.TileContext(nc) as tc:
        from contextlib import ExitStack
        with ExitStack() as ctx:
            consts = ctx.enter_context(tc.tile_pool(name="consts", bufs=1))
            xres = ctx.enter_context(tc.tile_pool(name="xres", bufs=1))
            pa = ctx.enter_context(tc.tile_pool(name="pa", bufs=3))
            pb = ctx.enter_context(tc.tile_pool(name="pb", bufs=3))
            pc = ctx.enter_context(tc.tile_pool(name="pc", bufs=3))
            psum = ctx.enter_context(tc.tile_pool(name="psum", bufs=2, space="PSUM"))

            # ---- constants ----
            def ld(name, ap_, shape):
                t = consts.tile(shape, dt.bfloat16, tag=name, name=name)
                nc.sync.dma_start(out=t, in_=ap_)
                return t

            t_chh = [ld(f"chh{j}", chh[j], [128, 258]) for j in range(2)]
            t_r1 = [ld(f"r1{j}", r1[j], [128, 258]) for j in range(2)]
            t_r2 = [ld(f"r2{j}", r2[j], [128, 258]) for j in range(2)]
            t_r2m = [ld(f"r2m{j}", r2m[j], [128, 258]) for j in range(2)]
            t_cr0 = ld("cr0", cr0, [128, 256])
            t_ci0 = ld("ci0", ci0, [128, 256])
            t_ni0 = ld("ni0", ni0, [128, 256])
            t_crM = ld("crM", crM, [128, 256])
            t_ciM = ld("ciM", ciM, [128, 256])
            t_niM = ld("niM", niM, [128, 256])
            t_gcx = ld("gcx", gcx, [128, 256])
            t_gsx = ld("gsx", gsx, [128, 256])
            t_w = {n: ld(n, ap_, [96, 96]) for n, ap_ in wts.items()}
            t_b1 = consts.tile([96, 2], dt.float32, tag="b1", name="t_b1")
            nc.sync.dma_start(out=t_b1, in_=b1c)
            t_bA = consts.tile([96, 2], dt.float32, tag="bA", name="t_bA")
            nc.sync.dma_start(out=t_bA, in_=bAc)
            t_bM = consts.tile([96, 2], dt.float32, tag="bM", name="t_bM")
            nc.sync.dma_start(out=t_bM, in_=bMc)

            # x resident tiles, one per (channel-group, h-chunk), reused across batches
            xr = [[xres.tile([128, CG, 256], dt.bfloat16, tag=f"xr{g}_{hc}",
                             name=f"xr{g}_{hc}") for hc in range(2)]
                  for g in range(NG)]

            # =================== Phase A ===================
            def emit_A(b):
                for g in range(NG):
                    c4 = g * CG
                    for hc in range(2):
                        nc.sync.dma_start(
                            out=xr[g][hc],
                            in_=xbf[b, c4:c4 + CG, hc * 128:(hc + 1) * 128, :]
                            .transpose([1, 0, 2]))
                    ztw = pa.tile([128, 2, CG, 258], dt.bfloat16, tag="ztw", name="ztw",
                                  bufs=2)
                    for cl in range(CG):
                        psY = psum.tile([128, 2, 258], dt.float32, tag="tP",
                                        name="psY", bufs=2, padded_shape=[128, 2, 512])
                        for wc in range(2):
                            for hc in range(2):
                                nc.tensor.matmul(
                                    psY[:, wc, :],
                                    lhsT=xr[g][hc][:, cl, wc * 128:(wc + 1) * 128],
                                    rhs=t_chh[hc], start=(hc == 0), stop=(hc == 1),
                                    skip_group_check=True)
                        yp = pa.tile([128, 2, 258], dt.bfloat16, tag="yp", name="yp", bufs=2)
                        nc.vector.tensor_copy(yp, psY)
                        ys = [yp[:, 0, :], yp[:, 1, :]]
                        psz = psum.tile([128, 2, 258], dt.float32, tag="tQ",
                                        name="psz", bufs=2, padded_shape=[128, 2, 512])
                        nc.tensor.matmul(psz[:, 0, :], lhsT=ys[0][:, 0:128], rhs=t_r1[0],
                                         start=True, stop=False, skip_group_check=True)
                        nc.tensor.matmul(psz[:, 0, :], lhsT=ys[0][:, 129:257], rhs=t_r2[0],
                                         start=False, stop=False, skip_group_check=True)
                        nc.tensor.matmul(psz[:, 0, :], lhsT=ys[1][:, 0:128], rhs=t_r1[1],
                                         start=False, stop=False, skip_group_check=True)
                        nc.tensor.matmul(psz[:, 0, :], lhsT=ys[1][:, 129:257], rhs=t_r2[1],
                                         start=False, stop=True, skip_group_check=True)
                        nc.tensor.matmul(psz[:, 1, :], lhsT=ys[0][:, 1:129], rhs=t_r1[0],
                                         start=True, stop=False, skip_group_check=True)
                        nc.tensor.matmul(psz[:, 1, :], lhsT=ys[0][:, 130:258], rhs=t_r2m[0],
                                         start=False, stop=False, skip_group_check=True)
                        nc.tensor.matmul(psz[:, 1, :], lhsT=ys[1][:, 1:129], rhs=t_r1[1],
                                         start=False, stop=False, skip_group_check=True)
                        nc.tensor.matmul(psz[:, 1, :], lhsT=ys[1][:, 130:258], rhs=t_r2m[1],
                                         start=False, stop=True, skip_group_check=True)
                        nc.scalar.copy(ztw[:, :, cl, :], psz)
                    nc.gpsimd.dma_start(out=zD[b][c4:c4 + CG, :, :].transpose([1, 0, 2]),
                                        in_=ztw[:, 0, :, :])
                    nc.gpsimd.dma_start(out=zM[b][c4:c4 + CG, :, :].transpose([1, 0, 2]),
                                        in_=ztw[:, 1, :, :])
                    yield

            # =================== Phase B ===================
            def emit_B(b):
                for half, (zsrc, sdst) in enumerate([(zD[b], sD[b]), (zM[b], sM[b])]):
                    for (u0, cnt) in b_groups():
                        zg = pb.tile([96, U, 258], dt.bfloat16, tag="zg", name="zg", bufs=3)
                        nc.sync.dma_start(out=zg[:, 0:cnt, :], in_=zsrc[:, u0:u0 + cnt, :])
                        zre = zg[:, 0:cnt, 0:NV]
                        zim = zg[:, 0:cnt, NV:258]
                        p1r = psum.tile([96, U, 130], dt.float32, tag="tP", name="p1r",
                                        bufs=2)
                        p1i = psum.tile([96, U, 130], dt.float32, tag="tQ", name="p1i",
                                        bufs=2)
                        nc.tensor.matmul(p1r[:, 0:cnt, 0:NV], lhsT=t_w['w1r'], rhs=zre,
                                         start=True, stop=False)
                        nc.tensor.matmul(p1r[:, 0:cnt, 0:NV], lhsT=t_w['w1in'], rhs=zim,
                                         start=False, stop=True)
                        nc.tensor.matmul(p1i[:, 0:cnt, 0:NV], lhsT=t_w['w1i'], rhs=zre,
                                         start=True, stop=False)
                        nc.tensor.matmul(p1i[:, 0:cnt, 0:NV], lhsT=t_w['w1r'], rhs=zim,
                                         start=False, stop=True)
                        o1r = pb.tile([96, U, 130], dt.bfloat16, tag="o1r", name="o1r", bufs=2)
                        o1i = pb.tile([96, U, 130], dt.bfloat16, tag="o1i", name="o1i", bufs=2)
                        nc.scalar.activation(o1r[:, 0:cnt, :], p1r[:, 0:cnt, :],
                                             Act.Relu, bias=t_b1[:, 0:1])
                        nc.scalar.activation(o1i[:, 0:cnt, :], p1i[:, 0:cnt, :],
                                             Act.Relu, bias=t_b1[:, 1:2])
                        p2r = psum.tile([96, U, 130], dt.float32, tag="tP", name="p2r",
                                        bufs=2)
                        p2i = psum.tile([96, U, 130], dt.float32, tag="tQ", name="p2i",
                                        bufs=2)
                        nc.tensor.matmul(p2r[:, 0:cnt, 0:NV], lhsT=t_w['w2r'],
                                         rhs=o1r[:, 0:cnt, 0:NV], start=True, stop=False)
                        nc.tensor.matmul(p2r[:, 0:cnt, 0:NV], lhsT=t_w['w2in'],
                                         rhs=o1i[:, 0:cnt, 0:NV], start=False, stop=True)
                        nc.tensor.matmul(p2i[:, 0:cnt, 0:NV], lhsT=t_w['w2i'],
                                         rhs=o1r[:, 0:cnt, 0:NV], start=True, stop=False)
                        nc.tensor.matmul(p2i[:, 0:cnt, 0:NV], lhsT=t_w['w2r'],
                                         rhs=o1i[:, 0:cnt, 0:NV], start=False, stop=True)
                        # softshrink s = relu(t+b2-lam) + min(t+b2+lam, 0), computed over
                        # [re(130) | im(130)] = main + nyquist + junk cols in one pass
                        sfA = pb.tile([96, U, 260], dt.bfloat16, tag="sA", name="sA", bufs=2)
                        sfM = pb.tile([96, U, 260], dt.bfloat16, tag="sM", name="sM", bufs=2)
                        stf = pb.tile([96, U, 260], dt.bfloat16, tag="stf", name="stf", bufs=2)
                        nc.scalar.activation(sfA[:, 0:cnt, 0:130], p2r[:, 0:cnt, :],
                                             Act.Relu, bias=t_bA[:, 0:1])
                        nc.scalar.activation(sfA[:, 0:cnt, 130:260], p2i[:, 0:cnt, :],
                                             Act.Relu, bias=t_bA[:, 1:2])
                        nc.vector.tensor_scalar(sfM[:, 0:cnt, 0:130], p2r[:, 0:cnt, :],
                                                t_bM[:, 0:1], 0.0, Alu.add, Alu.min)
                        nc.vector.tensor_scalar(sfM[:, 0:cnt, 130:260], p2i[:, 0:cnt, :],
                                                t_bM[:, 1:2], 0.0, Alu.add, Alu.min)
                        # stf row layout == sd row: [re-main(128) | nyim | pad |
                        #                            im-main(128) | -nyre | pad]
                        # big TT covers 0:258 (re-main correct; im cols land at 130:258)
                        nc.vector.tensor_tensor(stf[:, 0:cnt, 0:258], sfA[:, 0:cnt, 0:258],
                                                sfM[:, 0:cnt, 0:258], Alu.add)
                        # fixups: col128 <- s_im_ny (srcs col 258); col258 <- -s_re_ny
                        nc.vector.tensor_tensor(stf[:, 0:cnt, 128:129], sfA[:, 0:cnt, 258:259],
                                                sfM[:, 0:cnt, 258:259], Alu.add)
                        nc.vector.scalar_tensor_tensor(
                            stf[:, 0:cnt, 258:259], sfA[:, 0:cnt, 128:129], -1.0,
                            sfM[:, 0:cnt, 128:129], Alu.mult, Alu.subtract)
                        nc.gpsimd.dma_start(out=sdst[:, u0:u0 + cnt, :],
                                            in_=stf[:, 0:cnt, :])
                        yield

            # =================== Phase C ===================
            def emit_C(b):
                for g in range(NG):
                    c4 = g * CG
                    stD = pc.tile([128, CG, 260], dt.bfloat16, tag="stD", name="stD", bufs=2)
                    stM = pc.tile([128, CG, 260], dt.bfloat16, tag="stM", name="stM", bufs=2)
                    nc.sync.dma_start(out=stD, in_=sD[b][c4:c4 + CG, :, :].transpose([1, 0, 2]))
                    nc.sync.dma_start(out=stM, in_=sM[b][c4:c4 + CG, :, :].transpose([1, 0, 2]))
                    xcf = pc.tile([128, CG, 2, 256], dt.bfloat16, tag="xcf", name="xcf", bufs=2)
                    for hc in range(2):
                        nc.sync.dma_start(
                            out=xcf[:, :, hc, :],
                            in_=xbf[b, c4:c4 + CG, hc * 128:(hc + 1) * 128, :]
                            .transpose([1, 0, 2]))
                    otw = pc.tile([128, CG, 2, 256], dt.bfloat16, tag="otw", name="otw", bufs=2)
                    for cl in range(CG):
                        dd = stD[:, cl, :]
                        mm = stM[:, cl, :]
                        # P1 -> cols 0:256 ; P2 -> cols 256:512 of one psum bank
                        pP = psum.tile([128, 512], dt.float32, tag="tP", name="pP", bufs=2)
                        nc.tensor.matmul(pP[:, 0:256], lhsT=dd[:, 0:128], rhs=t_cr0,
                                         start=True, stop=False, skip_group_check=True)
                        nc.tensor.matmul(pP[:, 0:256], lhsT=dd[:, 130:258], rhs=t_ni0,
                                         start=False, stop=False, skip_group_check=True)
                        nc.tensor.matmul(pP[:, 0:256], lhsT=mm[:, 0:128], rhs=t_crM,
                                         start=False, stop=False, skip_group_check=True)
                        nc.tensor.matmul(pP[:, 0:256], lhsT=mm[:, 130:258], rhs=t_niM,
                                         start=False, stop=False, skip_group_check=True)
                        nc.tensor.matmul(pP[:, 256:512], lhsT=dd[:, 1:129], rhs=t_ci0,
                                         start=False, stop=False, skip_group_check=True)
                        nc.tensor.matmul(pP[:, 256:512], lhsT=dd[:, 131:259], rhs=t_cr0,
                                         start=False, stop=False, skip_group_check=True)
                        nc.tensor.matmul(pP[:, 256:512], lhsT=mm[:, 1:129], rhs=t_ciM,
                                         start=False, stop=False, skip_group_check=True)
                        nc.tensor.matmul(pP[:, 256:512], lhsT=mm[:, 131:259], rhs=t_crM,
                                         start=False, stop=True, skip_group_check=True)
                        psf = pc.tile([128, 512], dt.bfloat16, tag="psf", name="psf", bufs=2)
                        nc.scalar.copy(psf, pP)
                        pso = psum.tile([128, 512], dt.float32, tag="tQ", name="pso", bufs=2)
                        for hc in range(2):
                            os_ = slice(hc * 256, (hc + 1) * 256)
                            nc.tensor.matmul(pso[:, os_], lhsT=psf[:, hc * 128:(hc + 1) * 128],
                                             rhs=t_gcx, start=(hc == 0), stop=False,
                                             skip_group_check=True)
                            nc.tensor.matmul(pso[:, os_],
                                             lhsT=psf[:, 256 + hc * 128:256 + (hc + 1) * 128],
                                             rhs=t_gsx, start=False, stop=(hc == 1),
                                             skip_group_check=True)
                        nc.vector.tensor_tensor(otw[:, cl, :, :], pso, xcf[:, cl, :, :],
                                                Alu.add)
                    for hc in range(2):
                        nc.gpsimd.dma_start(
                            out=outd[b, c4:c4 + CG, hc * 128:(hc + 1) * 128, :]
                            .transpose([1, 0, 2]),
                            in_=otw[:, :, hc, :])
                    yield

            # =================== zipped schedule ===================
            def run_zip(gens, ratio):
                """Round-robin with per-gen step ratios until all exhausted."""
                done = [False] * len(gens)
                while not all(done):
                    for gi, gen in enumerate(gens):
                        if done[gi]:
                            continue
                        for _ in range(ratio[gi]):
                            try:
                                next(gen)
                            except StopIteration:
                                done[gi] = True
                                break

            for _ in emit_A(0):
                pass
            run_zip([emit_B(0), emit_A(1)], [4, 1])
            run_zip([emit_C(0), emit_B(1)], [1, 4])
            for _ in emit_C(1):
                pass

    nc.compile()
    return nc


_NC_CACHE = {}


def _get_nc():
    if 'nc' not in _NC_CACHE:
        _NC_CACHE['nc'] = build_nc()
    return _NC_CACHE['nc']


def make_in_maps(x, w1, b1, w2, b2):
    hc = make_host_consts()
    x = np.asarray(x, dtype=np.float32)
    in_maps = []
    for k in range(NCORES):
        xk = np.ascontiguousarray(x[:, BLK * k:BLK * (k + 1)]).astype(BF16)
        b1k = b1[k, :, 0, 0, :].astype(np.float32)
        b2k = b2[k, :, 0, 0, :].astype(np.float32)
        m = dict(
            xbf=xk,
            chh=hc['chh'], r1=hc['r1'], r2=hc['r2'], r2m=hc['r2m'],
            cr0=hc['cr0'], ci0=hc['ci0'], ni0=hc['ni0'],
            crM=hc['crM'], ciM=hc['ciM'], niM=hc['niM'],
            gcx=hc['gcx'], gsx=hc['gsx'],
            w1r=w1[k, :, :, 0].astype(BF16),
            w1i=w1[k, :, :, 1].astype(BF16),
            w1in=(-w1[k, :, :, 1]).astype(BF16),
            w2r=w2[k, :, :, 0].astype(BF16),
            w2i=w2[k, :, :, 1].astype(BF16),
            w2in=(-w2[k, :, :, 1]).astype(BF16),
            b1c=np.ascontiguousarray(b1k),
            bAc=np.ascontiguousarray(b2k - LAM),
            bMc=np.ascontiguousarray(b2k + LAM),
        )
        in_maps.append(m)
    return in_maps


def kernel(x, w1, b1, w2, b2):
    from concourse.bass_utils import run_bass_kernel_spmd
    nc = _get_nc()
    in_maps = make_in_maps(np.asarray(x), np.asarray(w1), np.asarray(b1),
                           np.asarray(w2), np.asarray(b2))
    res = run_bass_kernel_spmd(nc, in_maps, core_ids=list(range(NCORES)))
    outs = [res.results[k]['out'].astype(np.float32) for k in range(NCORES)]
    return np.concatenate(outs, axis=1)


# revision 19
# speedup vs baseline: 1.7438x; 1.0389x over previous
"""Trainium2 Bass kernel for DistributedAFNO2D (v2).

Problem: x(2,768,256,256) f32; per-block (8 blocks of 96 ch) spectral MLP:
  out = irfft2( softshrink( W2*relu(W1*rfft2(x) + b1) + b2 ) ) + x
Sharding: block k -> core k (8 cores). No collectives.

v2 design vs v1 (1.88ms):
 - Phase A (fwd FFT): S1 computes only u=0..128 (rfft symmetry), S2 derives
   the mirror half u=255..128 from conj(Y) with sign-flipped EW consts.
   Mirror rows live in their own DRAM buffer (zbufM, rows j <-> u=255-j);
   all reversals are absorbed into host-built constant row orders.
 - Phase B (spectral MLP): weight-stationary matmuls (96-col LDW hides under
   N=387 moving), 3 u-rows per group, re/im in separate PSUM tiles.
   softshrink = relu(t+b2-lam) + min(t+b2+lam, 0) via ACT+DVE.
 - Phase C (inv FFT): DC(v=0) folded into the main matmuls (P1 rows=Qr[0..127]
   with Gc[0]=1/16 row); Nyquist v=128 folded via P2 row127=-Qr[128] paired
   with -Gc[128] in GSX. No tiny DC matmuls, no scalar_tensor_tensor.
 - Per-batch DRAM tensors + zipped emission (A0 | B0+A1 | C0+B1 | C1) keep
   the PE warm; x stays resident in SBUF for the residual add.
 - Output bf16, upcast on host.

sd column layout (C1 lhsT windows, per u-row):
  [ s_re v0..127 | s_im v128 | s_im v0..127 | -s_re v128 ]
   P1-A=0:128 (x)CHIr ; P1-B=129:257 (x)-CHIi -> rows Qr v0..127
   P2-A=1:129 (x)CHIi ; P2-B=130:258 (x)CHIr  -> rows Qi v1..127, row127=-Qr[128]
"""
import sys
import numpy as np

sys.path.insert(0, "/opt/trn_rl_repo")

import ml_dtypes

BF16 = ml_dtypes.bfloat16

H = 256
W = 256
NV = 129
BLK = 96
NCORES = 8
B = 2
LAM = 0.01
U = 3  # u-rows per phase-B group
CG = 8  # channels per DMA batch


def b_groups():
    """(u0, cnt) groups covering 128 rows."""
    out = []
    u0 = 0
    while u0 < 128:
        cnt = min(U, 128 - u0)
        out.append((u0, cnt))
        u0 += cnt
    return out


def make_host_consts():
    I = np.eye(H, dtype=np.float64)
    F = np.fft.fft(I, axis=0, norm='ortho')       # F[u,h]
    Fi = np.fft.ifft(I, axis=0, norm='ortho')     # Fi[h,u]
    CHr = F.real.T.copy()                          # [h,u]
    CHi = F.imag.T.copy()
    EWr = F.real.T[:, :NV].copy()                  # [w,v]
    EWi = F.imag.T[:, :NV].copy()
    CHIr = Fi.real.T.copy()                        # [u,h]
    CHIi = Fi.imag.T.copy()
    Ir = np.eye(NV)
    Gc = np.fft.irfft(Ir, n=W, axis=-1, norm='ortho')        # [v,w]
    Gs = np.fft.irfft(1j * Ir, n=W, axis=-1, norm='ortho')   # [v,w]

    c = {}
    # A1: [h-chunk, (CHr u0..128 | CHi u0..128)]
    c['chh'] = np.stack([
        np.concatenate([CHr[j * 128:(j + 1) * 128, 0:NV],
                        CHi[j * 128:(j + 1) * 128, 0:NV]], axis=1)
        for j in range(2)])
    # A2 direct: R1 = [EWr|EWi], R2 = [-EWi|EWr]; mirror: R2m = [EWi|-EWr]
    c['r1'] = np.stack([
        np.concatenate([EWr[j * 128:(j + 1) * 128], EWi[j * 128:(j + 1) * 128]], axis=1)
        for j in range(2)])
    c['r2'] = np.stack([
        np.concatenate([-EWi[j * 128:(j + 1) * 128], EWr[j * 128:(j + 1) * 128]], axis=1)
        for j in range(2)])
    c['r2m'] = (-c['r2']).copy()
    # C1 rhs consts: direct rows u=0..127; mirror rows j <-> u=255-j
    perm = np.array([255 - j for j in range(128)])
    c['cr0'] = CHIr[0:128]
    c['ci0'] = CHIi[0:128]
    c['ni0'] = -CHIi[0:128]
    c['crM'] = CHIr[perm]
    c['ciM'] = CHIi[perm]
    c['niM'] = -CHIi[perm]
    # C2: GCX rows v=0..127 (incl DC); GSX rows = [Gs v1..127 ; -Gc v128]
    c['gcx'] = Gc[0:128]
    c['gsx'] = np.concatenate([Gs[1:128], -Gc[128:129]], axis=0)
    return {k: v.astype(BF16) for k, v in c.items()}


def build_nc():
    import concourse.bass as bass
    import concourse.tile as tile
    from concourse import bacc, mybir

    dt = mybir.dt
    Alu = mybir.AluOpType
    Act = mybir.ActivationFunctionType
    nc = bacc.Bacc("TRN2", target_bir_lowering=False, debug=False)

    # ---- I/O ----
    xbf = nc.dram_tensor("xbf", [B, BLK, H, W], dt.bfloat16, kind="ExternalInput").ap()
    chh = nc.dram_tensor("chh", [2, 128, 258], dt.bfloat16, kind="ExternalInput").ap()
    r1 = nc.dram_tensor("r1", [2, 128, 258], dt.bfloat16, kind="ExternalInput").ap()
    r2 = nc.dram_tensor("r2", [2, 128, 258], dt.bfloat16, kind="ExternalInput").ap()
    r2m = nc.dram_tensor("r2m", [2, 128, 258], dt.bfloat16, kind="ExternalInput").ap()
    cr0 = nc.dram_tensor("cr0", [128, 256], dt.bfloat16, kind="ExternalInput").ap()
    ci0 = nc.dram_tensor("ci0", [128, 256], dt.bfloat16, kind="ExternalInput").ap()
    ni0 = nc.dram_tensor("ni0", [128, 256], dt.bfloat16, kind="ExternalInput").ap()
    crM = nc.dram_tensor("crM", [128, 256], dt.bfloat16, kind="ExternalInput").ap()
    ciM = nc.dram_tensor("ciM", [128, 256], dt.bfloat16, kind="ExternalInput").ap()
    niM = nc.dram_tensor("niM", [128, 256], dt.bfloat16, kind="ExternalInput").ap()
    gcx = nc.dram_tensor("gcx", [128, 256], dt.bfloat16, kind="ExternalInput").ap()
    gsx = nc.dram_tensor("gsx", [128, 256], dt.bfloat16, kind="ExternalInput").ap()
    wts = {n: nc.dram_tensor(n, [96, 96], dt.bfloat16, kind="ExternalInput").ap()
           for n in ['w1r', 'w1i', 'w1in', 'w2r', 'w2i', 'w2in']}
    b1c = nc.dram_tensor("b1c", [96, 2], dt.float32, kind="ExternalInput").ap()
    bAc = nc.dram_tensor("bAc", [96, 2], dt.float32, kind="ExternalInput").ap()  # b2-lam
    bMc = nc.dram_tensor("bMc", [96, 2], dt.float32, kind="ExternalInput").ap()  # b2+lam
    outd = nc.dram_tensor("out", [B, BLK, H, W], dt.bfloat16, kind="ExternalOutput").ap()

    # DRAM scratch, separate tensors per batch to avoid cross-batch false deps
    # sd row layout (260 cols): [s_re v0..127 | s_im v128 | pad | s_im v0..127 |
    #                            -s_re v128 | pad]
    # C1 windows: P1-A=0:128 P2-A=1:129 P1-B=130:258 P2-B=131:259
    zD = [nc.dram_tensor(f"zD{b}", [BLK, 128, 258], dt.bfloat16).ap() for b in range(B)]
    zM = [nc.dram_tensor(f"zM{b}", [BLK, 128, 258], dt.bfloat16).ap() for b in range(B)]
    sD = [nc.dram_tensor(f"sD{b}", [BLK, 128, 260], dt.bfloat16).ap() for b in range(B)]
    sM = [nc.dram_tensor(f"sM{b}", [BLK, 128, 260], dt.bfloat16).ap() for b in range(B)]

    NG = BLK // CG  # 24 channel groups

    with tile.TileContext(nc) as tc:
        from contextlib import ExitStack
        with ExitStack() as ctx:
            consts = ctx.enter_context(tc.tile_pool(name="consts", bufs=1))
            xres = ctx.enter_context(tc.tile_pool(name="xres", bufs=1))
            pa = ctx.enter_context(tc.tile_pool(name="pa", bufs=3))
            pb = ctx.enter_context(tc.tile_pool(name="pb", bufs=3))
            pc = ctx.enter_context(tc.tile_pool(name="pc", bufs=3))
            psum = ctx.enter_context(tc.tile_pool(name="psum", bufs=2, space="PSUM"))

            # ---- constants ----
            def ld(name, ap_, shape):
                t = consts.tile(shape, dt.bfloat16, tag=name, name=name)
                nc.sync.dma_start(out=t, in_=ap_)
                return t

            t_chh = [ld(f"chh{j}", chh[j], [128, 258]) for j in range(2)]
            t_r1 = [ld(f"r1{j}", r1[j], [128, 258]) for j in range(2)]
            t_r2 = [ld(f"r2{j}", r2[j], [128, 258]) for j in range(2)]
            t_r2m = [ld(f"r2m{j}", r2m[j], [128, 258]) for j in range(2)]
            t_cr0 = ld("cr0", cr0, [128, 256])
            t_ci0 = ld("ci0", ci0, [128, 256])
            t_ni0 = ld("ni0", ni0, [128, 256])
            t_crM = ld("crM", crM, [128, 256])
            t_ciM = ld("ciM", ciM, [128, 256])
            t_niM = ld("niM", niM, [128, 256])
            t_gcx = ld("gcx", gcx, [128, 256])
            t_gsx = ld("gsx", gsx, [128, 256])
            t_w = {n: ld(n, ap_, [96, 96]) for n, ap_ in wts.items()}
            t_b1 = consts.tile([96, 2], dt.float32, tag="b1", name="t_b1")
            nc.sync.dma_start(out=t_b1, in_=b1c)
            t_bA = consts.tile([96, 2], dt.float32, tag="bA", name="t_bA")
            nc.sync.dma_start(out=t_bA, in_=bAc)
            t_bM = consts.tile([96, 2], dt.float32, tag="bM", name="t_bM")
            nc.sync.dma_start(out=t_bM, in_=bMc)

            # x resident tiles, one per (channel-group, h-chunk), reused across batches
            xr = [[xres.tile([128, CG, 256], dt.bfloat16, tag=f"xr{g}_{hc}",
                             name=f"xr{g}_{hc}") for hc in range(2)]
                  for g in range(NG)]

            # =================== Phase A ===================
            def emit_A(b):
                for g in range(NG):
                    c4 = g * CG
                    for hc in range(2):
                        nc.sync.dma_start(
                            out=xr[g][hc],
                            in_=xbf[b, c4:c4 + CG, hc * 128:(hc + 1) * 128, :]
                            .transpose([1, 0, 2]))
                    ztw = pa.tile([128, 2, CG, 258], dt.bfloat16, tag="ztw", name="ztw",
                                  bufs=2)
                    for cl in range(CG):
                        psY = psum.tile([128, 2, 258], dt.float32, tag="tP",
                                        name="psY", bufs=2, padded_shape=[128, 2, 512])
                        for wc in range(2):
                            for hc in range(2):
                                nc.tensor.matmul(
                                    psY[:, wc, :],
                                    lhsT=xr[g][hc][:, cl, wc * 128:(wc + 1) * 128],
                                    rhs=t_chh[hc], start=(hc == 0), stop=(hc == 1),
                                    skip_group_check=True)
                        yp = pa.tile([128, 2, 258], dt.bfloat16, tag="yp", name="yp", bufs=2)
                        nc.vector.tensor_copy(yp, psY)
                        ys = [yp[:, 0, :], yp[:, 1, :]]
                        psz = psum.tile([128, 2, 258], dt.float32, tag="tQ",
                                        name="psz", bufs=2, padded_shape=[128, 2, 512])
                        nc.tensor.matmul(psz[:, 0, :], lhsT=ys[0][:, 0:128], rhs=t_r1[0],
                                         start=True, stop=False, skip_group_check=True)
                        nc.tensor.matmul(psz[:, 0, :], lhsT=ys[0][:, 129:257], rhs=t_r2[0],
                                         start=False, stop=False, skip_group_check=True)
                        nc.tensor.matmul(psz[:, 0, :], lhsT=ys[1][:, 0:128], rhs=t_r1[1],
                                         start=False, stop=False, skip_group_check=True)
                        nc.tensor.matmul(psz[:, 0, :], lhsT=ys[1][:, 129:257], rhs=t_r2[1],
                                         start=False, stop=True, skip_group_check=True)
                        nc.tensor.matmul(psz[:, 1, :], lhsT=ys[0][:, 1:129], rhs=t_r1[0],
                                         start=True, stop=False, skip_group_check=True)
                        nc.tensor.matmul(psz[:, 1, :], lhsT=ys[0][:, 130:258], rhs=t_r2m[0],
                                         start=False, stop=False, skip_group_check=True)
                        nc.tensor.matmul(psz[:, 1, :], lhsT=ys[1][:, 1:129], rhs=t_r1[1],
                                         start=False, stop=False, skip_group_check=True)
                        nc.tensor.matmul(psz[:, 1, :], lhsT=ys[1][:, 130:258], rhs=t_r2m[1],
                                         start=False, stop=True, skip_group_check=True)
                        nc.scalar.copy(ztw[:, :, cl, :], psz)
                    nc.gpsimd.dma_start(out=zD[b][c4:c4 + CG, :, :].transpose([1, 0, 2]),
                                        in_=ztw[:, 0, :, :])
                    nc.gpsimd.dma_start(out=zM[b][c4:c4 + CG, :, :].transpose([1, 0, 2]),
                                        in_=ztw[:, 1, :, :])
                    yield

            # =================== Phase B ===================
            def emit_B(b):
                for half, (zsrc, sdst) in enumerate([(zD[b], sD[b]), (zM[b], sM[b])]):
                    for (u0, cnt) in b_groups():
                        zg = pb.tile([96, U, 258], dt.bfloat16, tag="zg", name="zg", bufs=4)
                        nc.sync.dma_start(out=zg[:, 0:cnt, :], in_=zsrc[:, u0:u0 + cnt, :])
                        zre = zg[:, 0:cnt, 0:NV]
                        zim = zg[:, 0:cnt, NV:258]
                        p1r = psum.tile([96, U, 130], dt.float32, tag="tP", name="p1r",
                                        bufs=2)
                        p1i = psum.tile([96, U, 130], dt.float32, tag="tQ", name="p1i",
                                        bufs=2)
                        nc.tensor.matmul(p1r[:, 0:cnt, 0:NV], lhsT=t_w['w1r'], rhs=zre,
                                         start=True, stop=False)
                        nc.tensor.matmul(p1r[:, 0:cnt, 0:NV], lhsT=t_w['w1in'], rhs=zim,
                                         start=False, stop=True)
                        nc.tensor.matmul(p1i[:, 0:cnt, 0:NV], lhsT=t_w['w1i'], rhs=zre,
                                         start=True, stop=False)
                        nc.tensor.matmul(p1i[:, 0:cnt, 0:NV], lhsT=t_w['w1r'], rhs=zim,
                                         start=False, stop=True)
                        o1r = pb.tile([96, U, 130], dt.bfloat16, tag="o1r", name="o1r", bufs=3)
                        o1i = pb.tile([96, U, 130], dt.bfloat16, tag="o1i", name="o1i", bufs=3)
                        nc.scalar.activation(o1r[:, 0:cnt, :], p1r[:, 0:cnt, :],
                                             Act.Relu, bias=t_b1[:, 0:1])
                        nc.scalar.activation(o1i[:, 0:cnt, :], p1i[:, 0:cnt, :],
                                             Act.Relu, bias=t_b1[:, 1:2])
                        p2r = psum.tile([96, U, 130], dt.float32, tag="tP", name="p2r",
                                        bufs=2)
                        p2i = psum.tile([96, U, 130], dt.float32, tag="tQ", name="p2i",
                                        bufs=2)
                        nc.tensor.matmul(p2r[:, 0:cnt, 0:NV], lhsT=t_w['w2r'],
                                         rhs=o1r[:, 0:cnt, 0:NV], start=True, stop=False)
                        nc.tensor.matmul(p2r[:, 0:cnt, 0:NV], lhsT=t_w['w2in'],
                                         rhs=o1i[:, 0:cnt, 0:NV], start=False, stop=True)
                        nc.tensor.matmul(p2i[:, 0:cnt, 0:NV], lhsT=t_w['w2i'],
                                         rhs=o1r[:, 0:cnt, 0:NV], start=True, stop=False)
                        nc.tensor.matmul(p2i[:, 0:cnt, 0:NV], lhsT=t_w['w2r'],
                                         rhs=o1i[:, 0:cnt, 0:NV], start=False, stop=True)
                        # softshrink s = relu(t+b2-lam) + min(t+b2+lam, 0), computed over
                        # [re(130) | im(130)] = main + nyquist + junk cols in one pass
                        sfA = pb.tile([96, U, 260], dt.bfloat16, tag="sA", name="sA", bufs=2)
                        sfM = pb.tile([96, U, 260], dt.bfloat16, tag="sM", name="sM", bufs=2)
                        stf = pb.tile([96, U, 260], dt.bfloat16, tag="stf", name="stf", bufs=3)
                        nc.scalar.activation(sfA[:, 0:cnt, 0:130], p2r[:, 0:cnt, :],
                                             Act.Relu, bias=t_bA[:, 0:1])
                        nc.scalar.activation(sfA[:, 0:cnt, 130:260], p2i[:, 0:cnt, :],
                                             Act.Relu, bias=t_bA[:, 1:2])
                        nc.vector.tensor_scalar(sfM[:, 0:cnt, 0:130], p2r[:, 0:cnt, :],
                                                t_bM[:, 0:1], 0.0, Alu.add, Alu.min)
                        nc.vector.tensor_scalar(sfM[:, 0:cnt, 130:260], p2i[:, 0:cnt, :],
                                                t_bM[:, 1:2], 0.0, Alu.add, Alu.min)
                        # stf row layout == sd row: [re-main(128) | nyim | pad |
                        #                            im-main(128) | -nyre | pad]
                        # big TT covers 0:258 (re-main correct; im cols land at 130:258)
                        nc.vector.tensor_tensor(stf[:, 0:cnt, 0:258], sfA[:, 0:cnt, 0:258],
                                                sfM[:, 0:cnt, 0:258], Alu.add)
                        # fixups: col128 <- s_im_ny (srcs col 258); col258 <- -s_re_ny
                        nc.vector.tensor_tensor(stf[:, 0:cnt, 128:129], sfA[:, 0:cnt, 258:259],
                                                sfM[:, 0:cnt, 258:259], Alu.add)
                        nc.vector.scalar_tensor_tensor(
                            stf[:, 0:cnt, 258:259], sfA[:, 0:cnt, 128:129], -1.0,
                            sfM[:, 0:cnt, 128:129], Alu.mult, Alu.subtract)
                        nc.gpsimd.dma_start(out=sdst[:, u0:u0 + cnt, :],
                                            in_=stf[:, 0:cnt, :])
                        yield

            # =================== Phase C ===================
            def emit_C(b):
                for g in range(NG):
                    c4 = g * CG
                    stD = pc.tile([128, CG, 260], dt.bfloat16, tag="stD", name="stD", bufs=2)
                    stM = pc.tile([128, CG, 260], dt.bfloat16, tag="stM", name="stM", bufs=2)
                    nc.sync.dma_start(out=stD, in_=sD[b][c4:c4 + CG, :, :].transpose([1, 0, 2]))
                    nc.sync.dma_start(out=stM, in_=sM[b][c4:c4 + CG, :, :].transpose([1, 0, 2]))
                    xcf = pc.tile([128, CG, 2, 256], dt.bfloat16, tag="xcf", name="xcf", bufs=2)
                    for hc in range(2):
                        nc.sync.dma_start(
                            out=xcf[:, :, hc, :],
                            in_=xbf[b, c4:c4 + CG, hc * 128:(hc + 1) * 128, :]
                            .transpose([1, 0, 2]))
                    otw = pc.tile([128, CG, 2, 256], dt.bfloat16, tag="otw", name="otw", bufs=2)
                    for cl in range(CG):
                        dd = stD[:, cl, :]
                        mm = stM[:, cl, :]
                        # P1 -> cols 0:256 ; P2 -> cols 256:512 of one psum bank
                        pP = psum.tile([128, 512], dt.float32, tag="tP", name="pP", bufs=2)
                        nc.tensor.matmul(pP[:, 0:256], lhsT=dd[:, 0:128], rhs=t_cr0,
                                         start=True, stop=False, skip_group_check=True)
                        nc.tensor.matmul(pP[:, 0:256], lhsT=dd[:, 130:258], rhs=t_ni0,
                                         start=False, stop=False, skip_group_check=True)
                        nc.tensor.matmul(pP[:, 0:256], lhsT=mm[:, 0:128], rhs=t_crM,
                                         start=False, stop=False, skip_group_check=True)
                        nc.tensor.matmul(pP[:, 0:256], lhsT=mm[:, 130:258], rhs=t_niM,
                                         start=False, stop=False, skip_group_check=True)
                        nc.tensor.matmul(pP[:, 256:512], lhsT=dd[:, 1:129], rhs=t_ci0,
                                         start=False, stop=False, skip_group_check=True)
                        nc.tensor.matmul(pP[:, 256:512], lhsT=dd[:, 131:259], rhs=t_cr0,
                                         start=False, stop=False, skip_group_check=True)
                        nc.tensor.matmul(pP[:, 256:512], lhsT=mm[:, 1:129], rhs=t_ciM,
                                         start=False, stop=False, skip_group_check=True)
                        nc.tensor.matmul(pP[:, 256:512], lhsT=mm[:, 131:259], rhs=t_crM,
                                         start=False, stop=True, skip_group_check=True)
                        psf = pc.tile([128, 512], dt.bfloat16, tag="psf", name="psf", bufs=2)
                        nc.scalar.copy(psf, pP)
                        pso = psum.tile([128, 512], dt.float32, tag="tQ", name="pso", bufs=2)
                        for hc in range(2):
                            os_ = slice(hc * 256, (hc + 1) * 256)
                            nc.tensor.matmul(pso[:, os_], lhsT=psf[:, hc * 128:(hc + 1) * 128],
                                             rhs=t_gcx, start=(hc == 0), stop=False,
                                             skip_group_check=True)
                            nc.tensor.matmul(pso[:, os_],
                                             lhsT=psf[:, 256 + hc * 128:256 + (hc + 1) * 128],
                                             rhs=t_gsx, start=False, stop=(hc == 1),
                                             skip_group_check=True)
                        nc.vector.tensor_tensor(otw[:, cl, :, :], pso, xcf[:, cl, :, :],
                                                Alu.add)
                    for hc in range(2):
                        nc.gpsimd.dma_start(
                            out=outd[b, c4:c4 + CG, hc * 128:(hc + 1) * 128, :]
                            .transpose([1, 0, 2]),
                            in_=otw[:, :, hc, :])
                    yield

            # =================== zipped schedule ===================
            def run_zip(gens, ratio):
                """Round-robin with per-gen step ratios until all exhausted."""
                done = [False] * len(gens)
                while not all(done):
                    for gi, gen in enumerate(gens):
                        if done[gi]:
                            continue
                        for _ in range(ratio[gi]):
                            try:
                                next(gen)
                            except StopIteration:
                                done[gi] = True
                                break

            for _ in emit_A(0):
                pass
            run_zip([emit_B(0), emit_A(1)], [4, 1])
            run_zip([emit_C(0), emit_B(1)], [1, 4])
            for _ in emit_C(1):
                pass

    nc.compile()
    return nc


_NC_CACHE = {}


def _get_nc():
    if 'nc' not in _NC_CACHE:
        _NC_CACHE['nc'] = build_nc()
    return _NC_CACHE['nc']


def make_in_maps(x, w1, b1, w2, b2):
    hc = make_host_consts()
    x = np.asarray(x, dtype=np.float32)
    in_maps = []
    for k in range(NCORES):
        xk = np.ascontiguousarray(x[:, BLK * k:BLK * (k + 1)]).astype(BF16)
        b1k = b1[k, :, 0, 0, :].astype(np.float32)
        b2k = b2[k, :, 0, 0, :].astype(np.float32)
        m = dict(
            xbf=xk,
            chh=hc['chh'], r1=hc['r1'], r2=hc['r2'], r2m=hc['r2m'],
            cr0=hc['cr0'], ci0=hc['ci0'], ni0=hc['ni0'],
            crM=hc['crM'], ciM=hc['ciM'], niM=hc['niM'],
            gcx=hc['gcx'], gsx=hc['gsx'],
            w1r=w1[k, :, :, 0].astype(BF16),
            w1i=w1[k, :, :, 1].astype(BF16),
            w1in=(-w1[k, :, :, 1]).astype(BF16),
            w2r=w2[k, :, :, 0].astype(BF16),
            w2i=w2[k, :, :, 1].astype(BF16),
            w2in=(-w2[k, :, :, 1]).astype(BF16),
            b1c=np.ascontiguousarray(b1k),
            bAc=np.ascontiguousarray(b2k - LAM),
            bMc=np.ascontiguousarray(b2k + LAM),
        )
        in_maps.append(m)
    return in_maps


def kernel(x, w1, b1, w2, b2):
    from concourse.bass_utils import run_bass_kernel_spmd
    nc = _get_nc()
    in_maps = make_in_maps(np.asarray(x), np.asarray(w1), np.asarray(b1),
                           np.asarray(w2), np.asarray(b2))
    res = run_bass_kernel_spmd(nc, in_maps, core_ids=list(range(NCORES)))
    outs = [res.results[k]['out'].astype(np.float32) for k in range(NCORES)]
    return np.concatenate(outs, axis=1)


# revision 20
# speedup vs baseline: 1.7584x; 1.0083x over previous
"""Trainium2 Bass kernel for DistributedAFNO2D (v2).

Problem: x(2,768,256,256) f32; per-block (8 blocks of 96 ch) spectral MLP:
  out = irfft2( softshrink( W2*relu(W1*rfft2(x) + b1) + b2 ) ) + x
Sharding: block k -> core k (8 cores). No collectives.

v2 design vs v1 (1.88ms):
 - Phase A (fwd FFT): S1 computes only u=0..128 (rfft symmetry), S2 derives
   the mirror half u=255..128 from conj(Y) with sign-flipped EW consts.
   Mirror rows live in their own DRAM buffer (zbufM, rows j <-> u=255-j);
   all reversals are absorbed into host-built constant row orders.
 - Phase B (spectral MLP): weight-stationary matmuls (96-col LDW hides under
   N=387 moving), 3 u-rows per group, re/im in separate PSUM tiles.
   softshrink = relu(t+b2-lam) + min(t+b2+lam, 0) via ACT+DVE.
 - Phase C (inv FFT): DC(v=0) folded into the main matmuls (P1 rows=Qr[0..127]
   with Gc[0]=1/16 row); Nyquist v=128 folded via P2 row127=-Qr[128] paired
   with -Gc[128] in GSX. No tiny DC matmuls, no scalar_tensor_tensor.
 - Per-batch DRAM tensors + zipped emission (A0 | B0+A1 | C0+B1 | C1) keep
   the PE warm; x stays resident in SBUF for the residual add.
 - Output bf16, upcast on host.

sd column layout (C1 lhsT windows, per u-row):
  [ s_re v0..127 | s_im v128 | s_im v0..127 | -s_re v128 ]
   P1-A=0:128 (x)CHIr ; P1-B=129:257 (x)-CHIi -> rows Qr v0..127
   P2-A=1:129 (x)CHIi ; P2-B=130:258 (x)CHIr  -> rows Qi v1..127, row127=-Qr[128]
"""
import sys
import numpy as np

sys.path.insert(0, "/opt/trn_rl_repo")

import ml_dtypes

BF16 = ml_dtypes.bfloat16

H = 256
W = 256
NV = 129
BLK = 96
NCORES = 8
B = 2
LAM = 0.01
U = 3  # u-rows per phase-B group
CG = 8  # channels per DMA batch


def b_groups():
    """(u0, cnt) groups covering 128 rows."""
    out = []
    u0 = 0
    while u0 < 128:
        cnt = min(U, 128 - u0)
        out.append((u0, cnt))
        u0 += cnt
    return out


def make_host_consts():
    I = np.eye(H, dtype=np.float64)
    F = np.fft.fft(I, axis=0, norm='ortho')       # F[u,h]
    Fi = np.fft.ifft(I, axis=0, norm='ortho')     # Fi[h,u]
    CHr = F.real.T.copy()                          # [h,u]
    CHi = F.imag.T.copy()
    EWr = F.real.T[:, :NV].copy()                  # [w,v]
    EWi = F.imag.T[:, :NV].copy()
    CHIr = Fi.real.T.copy()                        # [u,h]
    CHIi = Fi.imag.T.copy()
    Ir = np.eye(NV)
    Gc = np.fft.irfft(Ir, n=W, axis=-1, norm='ortho')        # [v,w]
    Gs = np.fft.irfft(1j * Ir, n=W, axis=-1, norm='ortho')   # [v,w]

    c = {}
    # A1: [h-chunk, (CHr u0..128 | CHi u0..128)]
    c['chh'] = np.stack([
        np.concatenate([CHr[j * 128:(j + 1) * 128, 0:NV],
                        CHi[j * 128:(j + 1) * 128, 0:NV]], axis=1)
        for j in range(2)])
    # A2 direct: R1 = [EWr|EWi], R2 = [-EWi|EWr]; mirror: R2m = [EWi|-EWr]
    c['r1'] = np.stack([
        np.concatenate([EWr[j * 128:(j + 1) * 128], EWi[j * 128:(j + 1) * 128]], axis=1)
        for j in range(2)])
    c['r2'] = np.stack([
        np.concatenate([-EWi[j * 128:(j + 1) * 128], EWr[j * 128:(j + 1) * 128]], axis=1)
        for j in range(2)])
    c['r2m'] = (-c['r2']).copy()
    # C1 rhs consts: direct rows u=0..127; mirror rows j <-> u=255-j
    perm = np.array([255 - j for j in range(128)])
    c['cr0'] = CHIr[0:128]
    c['ci0'] = CHIi[0:128]
    c['ni0'] = -CHIi[0:128]
    c['crM'] = CHIr[perm]
    c['ciM'] = CHIi[perm]
    c['niM'] = -CHIi[perm]
    # C2: GCX rows v=0..127 (incl DC); GSX rows = [Gs v1..127 ; -Gc v128]
    c['gcx'] = Gc[0:128]
    c['gsx'] = np.concatenate([Gs[1:128], -Gc[128:129]], axis=0)
    return {k: v.astype(BF16) for k, v in c.items()}


def build_nc():
    import concourse.bass as bass
    import concourse.tile as tile
    from concourse import bacc, mybir

    dt = mybir.dt
    Alu = mybir.AluOpType
    Act = mybir.ActivationFunctionType
    nc = bacc.Bacc("TRN2", target_bir_lowering=False, debug=False)

    # ---- I/O ----
    xbf = nc.dram_tensor("xbf", [B, BLK, H, W], dt.bfloat16, kind="ExternalInput").ap()
    chh = nc.dram_tensor("chh", [2, 128, 258], dt.bfloat16, kind="ExternalInput").ap()
    r1 = nc.dram_tensor("r1", [2, 128, 258], dt.bfloat16, kind="ExternalInput").ap()
    r2 = nc.dram_tensor("r2", [2, 128, 258], dt.bfloat16, kind="ExternalInput").ap()
    r2m = nc.dram_tensor("r2m", [2, 128, 258], dt.bfloat16, kind="ExternalInput").ap()
    cr0 = nc.dram_tensor("cr0", [128, 256], dt.bfloat16, kind="ExternalInput").ap()
    ci0 = nc.dram_tensor("ci0", [128, 256], dt.bfloat16, kind="ExternalInput").ap()
    ni0 = nc.dram_tensor("ni0", [128, 256], dt.bfloat16, kind="ExternalInput").ap()
    crM = nc.dram_tensor("crM", [128, 256], dt.bfloat16, kind="ExternalInput").ap()
    ciM = nc.dram_tensor("ciM", [128, 256], dt.bfloat16, kind="ExternalInput").ap()
    niM = nc.dram_tensor("niM", [128, 256], dt.bfloat16, kind="ExternalInput").ap()
    gcx = nc.dram_tensor("gcx", [128, 256], dt.bfloat16, kind="ExternalInput").ap()
    gsx = nc.dram_tensor("gsx", [128, 256], dt.bfloat16, kind="ExternalInput").ap()
    wts = {n: nc.dram_tensor(n, [96, 96], dt.bfloat16, kind="ExternalInput").ap()
           for n in ['w1r', 'w1i', 'w1in', 'w2r', 'w2i', 'w2in']}
    b1c = nc.dram_tensor("b1c", [96, 2], dt.float32, kind="ExternalInput").ap()
    bAc = nc.dram_tensor("bAc", [96, 2], dt.float32, kind="ExternalInput").ap()  # b2-lam
    bMc = nc.dram_tensor("bMc", [96, 2], dt.float32, kind="ExternalInput").ap()  # b2+lam
    outd = nc.dram_tensor("out", [B, BLK, H, W], dt.bfloat16, kind="ExternalOutput").ap()

    # DRAM scratch, separate tensors per batch to avoid cross-batch false deps
    # sd row layout (260 cols): [s_re v0..127 | s_im v128 | pad | s_im v0..127 |
    #                            -s_re v128 | pad]
    # C1 windows: P1-A=0:128 P2-A=1:129 P1-B=130:258 P2-B=131:259
    zD = [nc.dram_tensor(f"zD{b}", [BLK, 128, 258], dt.bfloat16).ap() for b in range(B)]
    zM = [nc.dram_tensor(f"zM{b}", [BLK, 128, 258], dt.bfloat16).ap() for b in range(B)]
    sD = [nc.dram_tensor(f"sD{b}", [BLK, 128, 260], dt.bfloat16).ap() for b in range(B)]
    sM = [nc.dram_tensor(f"sM{b}", [BLK, 128, 260], dt.bfloat16).ap() for b in range(B)]

    NG = BLK // CG  # 24 channel groups

    with tile.TileContext(nc) as tc:
        from contextlib import ExitStack
        with ExitStack() as ctx:
            consts = ctx.enter_context(tc.tile_pool(name="consts", bufs=1))
            xres = ctx.enter_context(tc.tile_pool(name="xres", bufs=1))
            pa = ctx.enter_context(tc.tile_pool(name="pa", bufs=3))
            pb = ctx.enter_context(tc.tile_pool(name="pb", bufs=3))
            pc = ctx.enter_context(tc.tile_pool(name="pc", bufs=3))
            psum = ctx.enter_context(tc.tile_pool(name="psum", bufs=2, space="PSUM"))

            # ---- constants ----
            def ld(name, ap_, shape):
                t = consts.tile(shape, dt.bfloat16, tag=name, name=name)
                nc.sync.dma_start(out=t, in_=ap_)
                return t

            t_chh = [ld(f"chh{j}", chh[j], [128, 258]) for j in range(2)]
            t_r1 = [ld(f"r1{j}", r1[j], [128, 258]) for j in range(2)]
            t_r2 = [ld(f"r2{j}", r2[j], [128, 258]) for j in range(2)]
            t_r2m = [ld(f"r2m{j}", r2m[j], [128, 258]) for j in range(2)]
            t_cr0 = ld("cr0", cr0, [128, 256])
            t_ci0 = ld("ci0", ci0, [128, 256])
            t_ni0 = ld("ni0", ni0, [128, 256])
            t_crM = ld("crM", crM, [128, 256])
            t_ciM = ld("ciM", ciM, [128, 256])
            t_niM = ld("niM", niM, [128, 256])
            t_gcx = ld("gcx", gcx, [128, 256])
            t_gsx = ld("gsx", gsx, [128, 256])
            t_w = {n: ld(n, ap_, [96, 96]) for n, ap_ in wts.items()}
            t_b1 = consts.tile([96, 2], dt.float32, tag="b1", name="t_b1")
            nc.sync.dma_start(out=t_b1, in_=b1c)
            t_bA = consts.tile([96, 2], dt.float32, tag="bA", name="t_bA")
            nc.sync.dma_start(out=t_bA, in_=bAc)
            t_bM = consts.tile([96, 2], dt.float32, tag="bM", name="t_bM")
            nc.sync.dma_start(out=t_bM, in_=bMc)

            # x resident tiles, one per (channel-group, h-chunk), reused across batches
            xr = [[xres.tile([128, CG, 256], dt.bfloat16, tag=f"xr{g}_{hc}",
                             name=f"xr{g}_{hc}") for hc in range(2)]
                  for g in range(NG)]

            # =================== Phase A ===================
            def emit_A(b):
                for g in range(NG):
                    c4 = g * CG
                    for hc in range(2):
                        nc.sync.dma_start(
                            out=xr[g][hc],
                            in_=xbf[b, c4:c4 + CG, hc * 128:(hc + 1) * 128, :]
                            .transpose([1, 0, 2]))
                    ztw = pa.tile([128, 2, CG, 258], dt.bfloat16, tag="ztw", name="ztw",
                                  bufs=2)
                    for cl in range(CG):
                        psY = psum.tile([128, 2, 258], dt.float32, tag="tA1",
                                        name="psY", bufs=1, padded_shape=[128, 2, 512])
                        for wc in range(2):
                            for hc in range(2):
                                nc.tensor.matmul(
                                    psY[:, wc, :],
                                    lhsT=xr[g][hc][:, cl, wc * 128:(wc + 1) * 128],
                                    rhs=t_chh[hc], start=(hc == 0), stop=(hc == 1),
                                    skip_group_check=True)
                        yp = pa.tile([128, 2, 258], dt.bfloat16, tag="yp", name="yp", bufs=2)
                        nc.vector.tensor_copy(yp, psY)
                        ys = [yp[:, 0, :], yp[:, 1, :]]
                        psz = psum.tile([128, 2, 258], dt.float32, tag="tA2",
                                        name="psz", bufs=1, padded_shape=[128, 2, 512])
                        nc.tensor.matmul(psz[:, 0, :], lhsT=ys[0][:, 0:128], rhs=t_r1[0],
                                         start=True, stop=False, skip_group_check=True)
                        nc.tensor.matmul(psz[:, 0, :], lhsT=ys[0][:, 129:257], rhs=t_r2[0],
                                         start=False, stop=False, skip_group_check=True)
                        nc.tensor.matmul(psz[:, 0, :], lhsT=ys[1][:, 0:128], rhs=t_r1[1],
                                         start=False, stop=False, skip_group_check=True)
                        nc.tensor.matmul(psz[:, 0, :], lhsT=ys[1][:, 129:257], rhs=t_r2[1],
                                         start=False, stop=True, skip_group_check=True)
                        nc.tensor.matmul(psz[:, 1, :], lhsT=ys[0][:, 1:129], rhs=t_r1[0],
                                         start=True, stop=False, skip_group_check=True)
                        nc.tensor.matmul(psz[:, 1, :], lhsT=ys[0][:, 130:258], rhs=t_r2m[0],
                                         start=False, stop=False, skip_group_check=True)
                        nc.tensor.matmul(psz[:, 1, :], lhsT=ys[1][:, 1:129], rhs=t_r1[1],
                                         start=False, stop=False, skip_group_check=True)
                        nc.tensor.matmul(psz[:, 1, :], lhsT=ys[1][:, 130:258], rhs=t_r2m[1],
                                         start=False, stop=True, skip_group_check=True)
                        nc.scalar.copy(ztw[:, :, cl, :], psz)
                    nc.gpsimd.dma_start(out=zD[b][c4:c4 + CG, :, :].transpose([1, 0, 2]),
                                        in_=ztw[:, 0, :, :])
                    nc.gpsimd.dma_start(out=zM[b][c4:c4 + CG, :, :].transpose([1, 0, 2]),
                                        in_=ztw[:, 1, :, :])
                    yield

            # =================== Phase B ===================
            def emit_B(b):
                for half, (zsrc, sdst) in enumerate([(zD[b], sD[b]), (zM[b], sM[b])]):
                    for (u0, cnt) in b_groups():
                        zg = pb.tile([96, U, 258], dt.bfloat16, tag="zg", name="zg", bufs=4)
                        nc.sync.dma_start(out=zg[:, 0:cnt, :], in_=zsrc[:, u0:u0 + cnt, :])
                        zre = zg[:, 0:cnt, 0:NV]
                        zim = zg[:, 0:cnt, NV:258]
                        p1r = psum.tile([96, U, 130], dt.float32, tag="tB1", name="p1r",
                                        bufs=2)
                        p1i = psum.tile([96, U, 130], dt.float32, tag="tB2", name="p1i",
                                        bufs=2)
                        nc.tensor.matmul(p1r[:, 0:cnt, 0:NV], lhsT=t_w['w1r'], rhs=zre,
                                         start=True, stop=False)
                        nc.tensor.matmul(p1r[:, 0:cnt, 0:NV], lhsT=t_w['w1in'], rhs=zim,
                                         start=False, stop=True)
                        nc.tensor.matmul(p1i[:, 0:cnt, 0:NV], lhsT=t_w['w1i'], rhs=zre,
                                         start=True, stop=False)
                        nc.tensor.matmul(p1i[:, 0:cnt, 0:NV], lhsT=t_w['w1r'], rhs=zim,
                                         start=False, stop=True)
                        o1r = pb.tile([96, U, 130], dt.bfloat16, tag="o1r", name="o1r", bufs=3)
                        o1i = pb.tile([96, U, 130], dt.bfloat16, tag="o1i", name="o1i", bufs=3)
                        nc.scalar.activation(o1r[:, 0:cnt, :], p1r[:, 0:cnt, :],
                                             Act.Relu, bias=t_b1[:, 0:1])
                        nc.scalar.activation(o1i[:, 0:cnt, :], p1i[:, 0:cnt, :],
                                             Act.Relu, bias=t_b1[:, 1:2])
                        p2r = psum.tile([96, U, 130], dt.float32, tag="tB1", name="p2r",
                                        bufs=2)
                        p2i = psum.tile([96, U, 130], dt.float32, tag="tB2", name="p2i",
                                        bufs=2)
                        nc.tensor.matmul(p2r[:, 0:cnt, 0:NV], lhsT=t_w['w2r'],
                                         rhs=o1r[:, 0:cnt, 0:NV], start=True, stop=False)
                        nc.tensor.matmul(p2r[:, 0:cnt, 0:NV], lhsT=t_w['w2in'],
                                         rhs=o1i[:, 0:cnt, 0:NV], start=False, stop=True)
                        nc.tensor.matmul(p2i[:, 0:cnt, 0:NV], lhsT=t_w['w2i'],
                                         rhs=o1r[:, 0:cnt, 0:NV], start=True, stop=False)
                        nc.tensor.matmul(p2i[:, 0:cnt, 0:NV], lhsT=t_w['w2r'],
                                         rhs=o1i[:, 0:cnt, 0:NV], start=False, stop=True)
                        # softshrink s = relu(t+b2-lam) + min(t+b2+lam, 0), computed over
                        # [re(130) | im(130)] = main + nyquist + junk cols in one pass
                        sfA = pb.tile([96, U, 260], dt.bfloat16, tag="sA", name="sA", bufs=2)
                        sfM = pb.tile([96, U, 260], dt.bfloat16, tag="sM", name="sM", bufs=2)
                        stf = pb.tile([96, U, 260], dt.bfloat16, tag="stf", name="stf", bufs=3)
                        nc.scalar.activation(sfA[:, 0:cnt, 0:130], p2r[:, 0:cnt, :],
                                             Act.Relu, bias=t_bA[:, 0:1])
                        nc.scalar.activation(sfA[:, 0:cnt, 130:260], p2i[:, 0:cnt, :],
                                             Act.Relu, bias=t_bA[:, 1:2])
                        nc.vector.tensor_scalar(sfM[:, 0:cnt, 0:130], p2r[:, 0:cnt, :],
                                                t_bM[:, 0:1], 0.0, Alu.add, Alu.min)
                        nc.vector.tensor_scalar(sfM[:, 0:cnt, 130:260], p2i[:, 0:cnt, :],
                                                t_bM[:, 1:2], 0.0, Alu.add, Alu.min)
                        # stf row layout == sd row: [re-main(128) | nyim | pad |
                        #                            im-main(128) | -nyre | pad]
                        # big TT covers 0:258 (re-main correct; im cols land at 130:258)
                        nc.vector.tensor_tensor(stf[:, 0:cnt, 0:258], sfA[:, 0:cnt, 0:258],
                                                sfM[:, 0:cnt, 0:258], Alu.add)
                        # fixups: col128 <- s_im_ny (srcs col 258); col258 <- -s_re_ny
                        nc.vector.tensor_tensor(stf[:, 0:cnt, 128:129], sfA[:, 0:cnt, 258:259],
                                                sfM[:, 0:cnt, 258:259], Alu.add)
                        nc.vector.scalar_tensor_tensor(
                            stf[:, 0:cnt, 258:259], sfA[:, 0:cnt, 128:129], -1.0,
                            sfM[:, 0:cnt, 128:129], Alu.mult, Alu.subtract)
                        nc.gpsimd.dma_start(out=sdst[:, u0:u0 + cnt, :],
                                            in_=stf[:, 0:cnt, :])
                        yield

            # =================== Phase C ===================
            def emit_C(b):
                for g in range(NG):
                    c4 = g * CG
                    stD = pc.tile([128, CG, 260], dt.bfloat16, tag="stD", name="stD", bufs=2)
                    stM = pc.tile([128, CG, 260], dt.bfloat16, tag="stM", name="stM", bufs=2)
                    nc.sync.dma_start(out=stD, in_=sD[b][c4:c4 + CG, :, :].transpose([1, 0, 2]))
                    nc.sync.dma_start(out=stM, in_=sM[b][c4:c4 + CG, :, :].transpose([1, 0, 2]))
                    xcf = pc.tile([128, CG, 2, 256], dt.bfloat16, tag="xcf", name="xcf", bufs=2)
                    for hc in range(2):
                        nc.sync.dma_start(
                            out=xcf[:, :, hc, :],
                            in_=xbf[b, c4:c4 + CG, hc * 128:(hc + 1) * 128, :]
                            .transpose([1, 0, 2]))
                    otw = pc.tile([128, CG, 2, 256], dt.bfloat16, tag="otw", name="otw", bufs=2)
                    for cl in range(CG):
                        dd = stD[:, cl, :]
                        mm = stM[:, cl, :]
                        # P1 -> cols 0:256 ; P2 -> cols 256:512 of one psum bank
                        pP = psum.tile([128, 512], dt.float32, tag="tA1", name="pP", bufs=1)
                        nc.tensor.matmul(pP[:, 0:256], lhsT=dd[:, 0:128], rhs=t_cr0,
                                         start=True, stop=False, skip_group_check=True)
                        nc.tensor.matmul(pP[:, 0:256], lhsT=dd[:, 130:258], rhs=t_ni0,
                                         start=False, stop=False, skip_group_check=True)
                        nc.tensor.matmul(pP[:, 0:256], lhsT=mm[:, 0:128], rhs=t_crM,
                                         start=False, stop=False, skip_group_check=True)
                        nc.tensor.matmul(pP[:, 0:256], lhsT=mm[:, 130:258], rhs=t_niM,
                                         start=False, stop=False, skip_group_check=True)
                        nc.tensor.matmul(pP[:, 256:512], lhsT=dd[:, 1:129], rhs=t_ci0,
                                         start=False, stop=False, skip_group_check=True)
                        nc.tensor.matmul(pP[:, 256:512], lhsT=dd[:, 131:259], rhs=t_cr0,
                                         start=False, stop=False, skip_group_check=True)
                        nc.tensor.matmul(pP[:, 256:512], lhsT=mm[:, 1:129], rhs=t_ciM,
                                         start=False, stop=False, skip_group_check=True)
                        nc.tensor.matmul(pP[:, 256:512], lhsT=mm[:, 131:259], rhs=t_crM,
                                         start=False, stop=True, skip_group_check=True)
                        psf = pc.tile([128, 512], dt.bfloat16, tag="psf", name="psf", bufs=2)
                        nc.scalar.copy(psf, pP)
                        pso = psum.tile([128, 512], dt.float32, tag="tA2", name="pso", bufs=1)
                        for hc in range(2):
                            os_ = slice(hc * 256, (hc + 1) * 256)
                            nc.tensor.matmul(pso[:, os_], lhsT=psf[:, hc * 128:(hc + 1) * 128],
                                             rhs=t_gcx, start=(hc == 0), stop=False,
                                             skip_group_check=True)
                            nc.tensor.matmul(pso[:, os_],
                                             lhsT=psf[:, 256 + hc * 128:256 + (hc + 1) * 128],
                                             rhs=t_gsx, start=False, stop=(hc == 1),
                                             skip_group_check=True)
                        nc.vector.tensor_tensor(otw[:, cl, :, :], pso, xcf[:, cl, :, :],
                                                Alu.add)
                    for hc in range(2):
                        nc.gpsimd.dma_start(
                            out=outd[b, c4:c4 + CG, hc * 128:(hc + 1) * 128, :]
                            .transpose([1, 0, 2]),
                            in_=otw[:, :, hc, :])
                    yield

            # =================== zipped schedule ===================
            def run_zip(gens, ratio):
                """Round-robin with per-gen step ratios until all exhausted."""
                done = [False] * len(gens)
                while not all(done):
                    for gi, gen in enumerate(gens):
                        if done[gi]:
                            continue
                        for _ in range(ratio[gi]):
                            try:
                                next(gen)
                            except StopIteration:
                                done[gi] = True
                                break

            for _ in emit_A(0):
                pass
            run_zip([emit_B(0), emit_A(1)], [4, 1])
            run_zip([emit_C(0), emit_B(1)], [1, 4])
            for _ in emit_C(1):
                pass

    nc.compile()
    return nc


_NC_CACHE = {}


def _get_nc():
    if 'nc' not in _NC_CACHE:
        _NC_CACHE['nc'] = build_nc()
    return _NC_CACHE['nc']


def make_in_maps(x, w1, b1, w2, b2):
    hc = make_host_consts()
    x = np.asarray(x, dtype=np.float32)
    in_maps = []
    for k in range(NCORES):
        xk = np.ascontiguousarray(x[:, BLK * k:BLK * (k + 1)]).astype(BF16)
        b1k = b1[k, :, 0, 0, :].astype(np.float32)
        b2k = b2[k, :, 0, 0, :].astype(np.float32)
        m = dict(
            xbf=xk,
            chh=hc['chh'], r1=hc['r1'], r2=hc['r2'], r2m=hc['r2m'],
            cr0=hc['cr0'], ci0=hc['ci0'], ni0=hc['ni0'],
            crM=hc['crM'], ciM=hc['ciM'], niM=hc['niM'],
            gcx=hc['gcx'], gsx=hc['gsx'],
            w1r=w1[k, :, :, 0].astype(BF16),
            w1i=w1[k, :, :, 1].astype(BF16),
            w1in=(-w1[k, :, :, 1]).astype(BF16),
            w2r=w2[k, :, :, 0].astype(BF16),
            w2i=w2[k, :, :, 1].astype(BF16),
            w2in=(-w2[k, :, :, 1]).astype(BF16),
            b1c=np.ascontiguousarray(b1k),
            bAc=np.ascontiguousarray(b2k - LAM),
            bMc=np.ascontiguousarray(b2k + LAM),
        )
        in_maps.append(m)
    return in_maps


def kernel(x, w1, b1, w2, b2):
    from concourse.bass_utils import run_bass_kernel_spmd
    nc = _get_nc()
    in_maps = make_in_maps(np.asarray(x), np.asarray(w1), np.asarray(b1),
                           np.asarray(w2), np.asarray(b2))
    res = run_bass_kernel_spmd(nc, in_maps, core_ids=list(range(NCORES)))
    outs = [res.results[k]['out'].astype(np.float32) for k in range(NCORES)]
    return np.concatenate(outs, axis=1)
